# revision 4
# baseline (speedup 1.0000x reference)
"""Trainium2 Bass kernel for nn_CNN_Comp_29240137351522 (dense_cnn).

Math:  y = |IFFT_N( FFT_N(x)^2 * C )[255:2303]|,  C = FFT_N(w0)^2 * FFT_N(wl) / N
with N = 2560 = 128*20 so the chained full convolutions are exact.

v2 changes vs baseline:
  - host pre-transposes x (feature-major), eliminating on-device transposes
    and their PSUM evictions
  - plane-combined PSUM tiles [*, 1024] = (half, plane, b-256) so each
    PSUM->SBUF eviction is ONE wide engine op instead of two
  - bf16 intermediates from F1 output onward (Abig, Zr/Pt, Ubig, u2) for
    2x DVE math and half-size pivot DMAs
  - square/magnitude math on DVE in bf16 (2x mode), balanced against ACT
  - stores split to Pool SWDGE; loads/pivots on SP HWDGE
"""

import numpy as np
import ml_dtypes

import concourse.bass as bass
import concourse.bacc as bacc
import concourse.mybir as mybir
from concourse.tile import TileContext
from concourse.bass_utils import run_bass_kernel_spmd

# ---------------- static problem config ----------------
B, NX = 4096, 1024
K0, KL = 129, 257
N = 2560
N1, N2 = 128, 20
NCORES = 8
BCORE = B // NCORES          # 512
CHUNK = 256
NCHUNKS = BCORE // CHUNK     # 2
N2OUT = 17                   # n2 in [1,18)
CROP0 = 255
CLASS_NUM = 2048
IBLK_I2 = (6, 6, 4)
JOFS_I2 = (0, 6, 12)
YRAW_ROWS = 8 * sum(IBLK_I2) * N2OUT  # 2176

f32 = mybir.dt.float32
f32r = mybir.dt.float32r
bf16 = mybir.dt.bfloat16
AO = mybir.AluOpType
AF = mybir.ActivationFunctionType


def _w(num, den):
    return np.exp(-2j * np.pi * np.asarray(num, np.float64) / den)


# ---------------- host-side constant arrays ----------------
def _build_consts():
    c = {}
    n1g = np.arange(N1)
    k1g = np.arange(N1)
    k2g = np.arange(N2)
    n2g8 = np.arange(8)

    # F1 lhsT: [128, 640]; block (g,jj) at partitions [32jj,32jj+32), cols [80g,80g+80)
    # rows (il in 4)*8 + n2, cols il*20 + k2; value W20[n2,k2] * W2560^{n1 k2}, n1=16g+4jj+il
    f1 = np.zeros((128, 640), np.complex128)
    for g in range(8):
        for jj in range(4):
            for il in range(4):
                n1 = 16 * g + 4 * jj + il
                blk = _w(np.outer(n2g8, k2g), N2) * _w(n1 * k2g, N)[None, :]
                f1[32 * jj + il * 8 : 32 * jj + il * 8 + 8, 80 * g + il * 20 : 80 * g + (il + 1) * 20] = blk
    c["cf1r"] = f1.real.astype(np.float32)
    c["cf1i"] = f1.imag.astype(np.float32)
    c["cf1n"] = (-f1.imag).astype(np.float32)

    # F3 lhsT (shared, f32r): W128[n1,k1] -- also used by the weight-DFT
    w3 = _w(np.outer(n1g, k1g), N1)
    c["cwfr"] = w3.real.astype(np.float32)
    c["cwfi"] = w3.imag.astype(np.float32)
    c["cwfn"] = (-w3.imag).astype(np.float32)

    # I1 base: W128i[k1,n1] (bf16, G built on device)
    wi = _w(-np.outer(k1g, n1g), N1)
    c["cwir"] = wi.real.astype(ml_dtypes.bfloat16)
    c["cwii"] = wi.imag.astype(ml_dtypes.bfloat16)

    # I2 lhsT: [120, 2176]; per (g,j) cols [off,off+M_j); block-diag il:
    # rows il*20+k2, cols il*17+(n2-1); value W20^{-k2 n2} * W2560^{-n1 k2}
    n2out = np.arange(1, 18)
    i2 = np.zeros((120, 2176), np.complex128)
    off = 0
    for g in range(8):
        for j, cnt in enumerate(IBLK_I2):
            for il in range(cnt):
                n1 = 16 * g + JOFS_I2[j] + il
                blk = _w(-np.outer(k2g, n2out), N2) * _w(-n1 * k2g, N)[:, None]
                i2[il * 20 : (il + 1) * 20, off + il * 17 : off + (il + 1) * 17] = blk
            off += cnt * N2OUT
    c["ci2r"] = i2.real.astype(ml_dtypes.bfloat16)
    c["ci2i"] = i2.imag.astype(ml_dtypes.bfloat16)
    c["ci2n"] = (-i2.imag).astype(ml_dtypes.bfloat16)

    # weight-DFT rhs constants
    nh = np.arange(128)
    t129 = _w(np.outer(nh, k2g), N)
    c["ct1r"] = t129.real.astype(np.float32)
    c["ct1i"] = t129.imag.astype(np.float32)
    t257b = _w(np.outer(nh, k2g), N) * _w(k2g, 20)[None, :]
    c["ct2r"] = t257b.real.astype(np.float32)
    c["ct2i"] = t257b.imag.astype(np.float32)
    t129e = _w(k2g, 20)
    c["te1r"] = t129e.real.astype(np.float32).reshape(1, N2)
    c["te1i"] = t129e.imag.astype(np.float32).reshape(1, N2)
    t257e = _w(k2g, 10)
    c["te2r"] = t257e.real.astype(np.float32).reshape(1, N2)
    c["te2i"] = t257e.imag.astype(np.float32).reshape(1, N2)

    c["ones1"] = np.ones((1, 128), np.float32)
    return c


CONSTS = _build_consts()


def host_x_perm():
    """perm[g*128 + i*8 + n2] = n2*128 + 16g + i"""
    perm = np.empty(NX, np.int64)
    for g in range(8):
        for i in range(16):
            for n2 in range(8):
                perm[g * 128 + i * 8 + n2] = n2 * 128 + 16 * g + i
    return perm


def yraw_maps():
    """row r of yraw -> output column (n-255), valid mask."""
    rows = []
    for g in range(8):
        for j, cnt in enumerate(IBLK_I2):
            for il in range(cnt):
                n1 = 16 * g + JOFS_I2[j] + il
                for q in range(N2OUT):
                    rows.append((q + 1) * 128 + n1)
    narr = np.array(rows)
    valid = (narr >= CROP0) & (narr < CROP0 + CLASS_NUM)
    return narr, valid


XPERM = host_x_perm()
YN, YVALID = yraw_maps()


def _ap(tile, dims, extra_off=0):
    return bass.AP(tile.tensor, tile[:].offset + extra_off, dims)


DEBUG_TAPS = False
AG_DT = bf16  # flip to f32r to debug precision/corruption
XP_BUFS = 2
GP_BUFS = 2
SP_BUFS = 2
EVICT_MODE = "wide-alt"
GSCALE_DVE = False


# ---------------- bass kernel builder ----------------
def build_nc():
    nc = bacc.Bacc("TRN2", target_bir_lowering=False, debug=False, num_devices=NCORES)

    # DRAM tensors: xh = pre-transposed permuted x, [NX, BCORE]
    d = {}
    d["xh_r"] = nc.dram_tensor("xh_r", [NX, BCORE], f32r, kind="ExternalInput")
    d["xh_i"] = nc.dram_tensor("xh_i", [NX, BCORE], f32r, kind="ExternalInput")
    for nm, shape in [("w0r", [K0]), ("w0i", [K0]), ("wlr", [KL]), ("wli", [KL])]:
        d[nm] = nc.dram_tensor(nm, shape, f32, kind="ExternalInput")
    cdt = {"cf1r": f32r, "cf1i": f32r, "cf1n": f32r,
           "cwfr": f32r, "cwfi": f32r, "cwfn": f32r,
           "cwir": bf16, "cwii": bf16,
           "ci2r": bf16, "ci2i": bf16, "ci2n": bf16,
           "ones1": f32r}
    for nm, arr in CONSTS.items():
        d[nm] = nc.dram_tensor(nm, list(arr.shape), cdt.get(nm, f32), kind="ExternalInput")
    yraw = nc.dram_tensor("yraw", [YRAW_ROWS, BCORE], bf16, kind="ExternalOutput")
    dbg = {}
    if DEBUG_TAPS:
        for nm, shape, ddt in [("dag", [80, 2048], AG_DT), ("dagf", [80, 2048], f32), ("dAbig", [128, 10240], AG_DT), ("dZr", [128, 5120], bf16),
                               ("dPt", [128, 5120], bf16), ("dUbig", [128, 10240], bf16),
                               ("du2", [120, 24 * 512], bf16)]:
            dbg[nm] = nc.dram_tensor(nm, shape, ddt, kind="ExternalOutput")

    with TileContext(nc) as tc:
        with (
            tc.tile_pool(name="cp", bufs=1) as cp,         # consts + persistent
            tc.tile_pool(name="bp", bufs=1) as bp,         # big single-buffer tiles
            tc.tile_pool(name="xp", bufs=1) as xp,         # chunk input tiles
            tc.tile_pool(name="gp", bufs=GP_BUFS) as gp,
            tc.tile_pool(name="zp", bufs=4) as zp,
            tc.tile_pool(name="qp", bufs=3) as qp,         # ag staging
            tc.tile_pool(name="sp", bufs=SP_BUFS) as sp,         # small rotating scratch
            tc.tile_pool(name="tp", bufs=3) as tp,         # f32 tmp tiles (weight prep)
            tc.tile_pool(name="psa", bufs=2, space="PSUM") as psa,  # 2 tags x 2 bufs x 2 banks
        ):
            # ---- input loads (halves) interleaved with critical consts ----
            xt_r = xp.tile([128, 4096], f32r, tag="xtr", name="xt_r")
            xt_i = xp.tile([128, 4096], f32r, tag="xti", name="xt_i")
            def load_quarter(q):
                for xt, srcnm in [(xt_r, "xh_r"), (xt_i, "xh_i")]:
                    sap = d[srcnm][:, :]
                    nc.sync.dma_start(
                        out=_ap(xt, [[4096, 128], [512, 2], [1, 512]],
                                extra_off=q * 1024),
                        in_=bass.AP(sap.tensor, sap.offset + q * 2 * 128 * BCORE,
                                    [[BCORE, 128], [128 * BCORE, 2], [1, 512]]),
                    )
            def load_half(h):
                load_quarter(2 * h)
                load_quarter(2 * h + 1)
            load_quarter(0)

            ct = {}
            def load_consts(names, eng):
                for nm in names:
                    arr = CONSTS[nm]
                    t = cp.tile(list(arr.shape), cdt.get(nm, f32), tag=nm, name=nm)
                    eng.dma_start(out=t[:], in_=d[nm][:, :] if arr.ndim == 2 else d[nm][:])
                    ct[nm] = t
            load_consts(["cf1r", "cf1i", "cf1n"], nc.sync)
            load_quarter(1)
            load_consts(["ct1r", "ct1i", "ct2r", "ct2i",
                         "te1r", "te1i", "te2r", "te2i", "ones1"], nc.gpsimd)
            load_consts(["cwfr", "cwfi", "cwfn", "cwir", "cwii",
                         "ci2r", "ci2i", "ci2n"], nc.gpsimd)

            # ---- load w0/wl pieces as [128,1] / [1,1] columns ----
            wc = {}
            for nm, src, lo, hi in [
                ("w0r_c", "w0r", 0, 128), ("w0i_c", "w0i", 0, 128),
                ("wlr_c1", "wlr", 0, 128), ("wli_c1", "wli", 0, 128),
                ("wlr_c2", "wlr", 128, 256), ("wli_c2", "wli", 128, 256),
            ]:
                t = cp.tile([128, 1], f32, tag=nm, name=nm)
                nc.sync.dma_start(out=t[:], in_=d[src][lo:hi])
                wc[nm] = t
            for nm, src, pos in [("w0r_e", "w0r", 128), ("w0i_e", "w0i", 128),
                                 ("wlr_e", "wlr", 256), ("wli_e", "wli", 256)]:
                t = cp.tile([1, 1], f32, tag=nm, name=nm)
                nc.sync.dma_start(out=t[:], in_=d[src][pos:pos + 1])
                wc[nm] = t
            load_half(1)

            # ---- weight DFT: W0, WL [128, 20] (f32 path, tiny) ----
            def build_rhs(tr, ti, cr_, ci_, out_r, out_i):
                tmp = tp.tile([tr.shape[0], N2], f32, tag="wtmp", name="wtmp")
                nc.vector.tensor_scalar(tmp[:], ti[:], ci_[:], None, AO.mult)
                nc.vector.scalar_tensor_tensor(out_r[:], tr[:], cr_[:], tmp[:], AO.mult, AO.subtract)
                tmp2 = tp.tile([tr.shape[0], N2], f32, tag="wtmp2", name="wtmp2")
                nc.vector.tensor_scalar(tmp2[:], tr[:], ci_[:], None, AO.mult)
                nc.vector.scalar_tensor_tensor(out_i[:], ti[:], cr_[:], tmp2[:], AO.mult, AO.add)

            def weight_dft(chunks, tail, out_r, out_i):
                ps_r = psa.tile([128, N2], f32, tag="pB", name="wpsr")
                ps_i = psa.tile([128, N2], f32, tag="pB", name="wpsi")
                rhs = []
                for (t_r, t_i, colr, coli) in chunks:
                    rr = tp.tile([128, N2], f32r, tag="wrhs_r", name="wrhs_r")
                    ri = tp.tile([128, N2], f32r, tag="wrhs_i", name="wrhs_i")
                    build_rhs(t_r, t_i, colr, coli, rr, ri)
                    rhs.append((rr, ri))
                te_r, te_i, er, ei = tail
                tr = tp.tile([1, N2], f32r, tag="wtail_r", name="wtail_r")
                ti_ = tp.tile([1, N2], f32r, tag="wtail_i", name="wtail_i")
                tmp = tp.tile([1, N2], f32, tag="wtmp3", name="wtmp3")
                nc.vector.tensor_scalar(tmp[:], te_i[:], ei[:], None, AO.mult)
                nc.vector.scalar_tensor_tensor(tr[:], te_r[:], er[:], tmp[:], AO.mult, AO.subtract)
                tmp2 = tp.tile([1, N2], f32, tag="wtmp4", name="wtmp4")
                nc.vector.tensor_scalar(tmp2[:], te_r[:], ei[:], None, AO.mult)
                nc.vector.scalar_tensor_tensor(ti_[:], te_i[:], er[:], tmp2[:], AO.mult, AO.add)
                first = True
                for (rr, ri) in rhs:
                    nc.tensor.matmul(ps_r[:], ct["cwfr"][:], rr[:], start=first, stop=False)
                    nc.tensor.matmul(ps_r[:], ct["cwfn"][:], ri[:], start=False, stop=False)
                    first = False
                nc.tensor.matmul(ps_r[:], ct["ones1"][:1, :], tr[:], start=False, stop=True)
                first = True
                for (rr, ri) in rhs:
                    nc.tensor.matmul(ps_i[:], ct["cwfi"][:], rr[:], start=first, stop=False)
                    nc.tensor.matmul(ps_i[:], ct["cwfr"][:], ri[:], start=False, stop=False)
                    first = False
                nc.tensor.matmul(ps_i[:], ct["ones1"][:1, :], ti_[:], start=False, stop=True)
                nc.vector.tensor_copy(out_r[:], ps_r[:])
                nc.vector.tensor_copy(out_i[:], ps_i[:])

            W0r = cp.tile([128, N2], f32, tag="W0r", name="W0r")
            W0i = cp.tile([128, N2], f32, tag="W0i", name="W0i")
            weight_dft(
                [(ct["ct1r"], ct["ct1i"], wc["w0r_c"], wc["w0i_c"])],
                (ct["te1r"], ct["te1i"], wc["w0r_e"], wc["w0i_e"]),
                W0r, W0i,
            )
            WLr = cp.tile([128, N2], f32, tag="WLr", name="WLr")
            WLi = cp.tile([128, N2], f32, tag="WLi", name="WLi")
            weight_dft(
                [(ct["ct1r"], ct["ct1i"], wc["wlr_c1"], wc["wli_c1"]),
                 (ct["ct2r"], ct["ct2i"], wc["wlr_c2"], wc["wli_c2"])],
                (ct["te2r"], ct["te2i"], wc["wlr_e"], wc["wli_e"]),
                WLr, WLi,
            )

            # ---- C = W0^2 * WL / N  [128, 20] ----
            Cr = cp.tile([128, N2], f32, tag="Cr", name="Cr")
            Ci = cp.tile([128, N2], f32, tag="Ci", name="Ci")
            ta = tp.tile([128, N2], f32, tag="ca", name="ca")
            tb = tp.tile([128, N2], f32, tag="cb", name="cb")
            tm1 = tp.tile([128, N2], f32, tag="cm1", name="cm1")
            tm2 = tp.tile([128, N2], f32, tag="cm2", name="cm2")
            nc.vector.tensor_mul(tm1[:], W0r[:], W0r[:])
            nc.vector.tensor_mul(tm2[:], W0i[:], W0i[:])
            nc.vector.tensor_sub(ta[:], tm1[:], tm2[:])
            nc.vector.tensor_mul(tm1[:], W0r[:], W0i[:])
            nc.vector.tensor_add(tb[:], tm1[:], tm1[:])
            nc.vector.tensor_mul(tm1[:], ta[:], WLr[:])
            nc.vector.tensor_mul(tm2[:], tb[:], WLi[:])
            nc.vector.tensor_sub(tm1[:], tm1[:], tm2[:])
            nc.vector.tensor_scalar(Cr[:], tm1[:], 1.0 / N, None, AO.mult)
            nc.vector.tensor_mul(tm1[:], ta[:], WLi[:])
            nc.vector.tensor_mul(tm2[:], tb[:], WLr[:])
            nc.vector.tensor_add(tm1[:], tm1[:], tm2[:])
            nc.vector.tensor_scalar(Ci[:], tm1[:], 1.0 / N, None, AO.mult)

            # ---- G variants (bf16): G_k2 = C[:,k2] row-scaled W128i ----
            Gr = cp.tile([128, N2 * 128], bf16, tag="Gr", name="Gr")
            Gi = cp.tile([128, N2 * 128], bf16, tag="Gi", name="Gi")
            Gn = cp.tile([128, N2 * 128], bf16, tag="Gn", name="Gn")   # -Gi
            def emit_g(k2):
                """Build G variants for one k2; alternates ts ops ACT/DVE."""
                cr_ = Cr[:, k2 : k2 + 1]
                ci_ = Ci[:, k2 : k2 + 1]
                sl = slice(k2 * 128, (k2 + 1) * 128)
                gt = tp.tile([128, 128], bf16, tag="gtmp", name="gtmp")
                gt2 = tp.tile([128, 128], bf16, tag="gtmp2", name="gtmp2")
                if k2 % 2 == 0:
                    nc.scalar.activation(gt[:], ct["cwii"][:], AF.Copy, scale=ci_)
                    nc.scalar.activation(gt2[:], ct["cwir"][:], AF.Copy, scale=ci_)
                else:
                    nc.vector.tensor_scalar(gt[:], ct["cwii"][:], ci_, None, AO.mult)
                    nc.vector.tensor_scalar(gt2[:], ct["cwir"][:], ci_, None, AO.mult)
                nc.vector.scalar_tensor_tensor(Gr[:, sl], ct["cwir"][:], cr_, gt[:], AO.mult, AO.subtract)
                nc.vector.scalar_tensor_tensor(Gi[:, sl], ct["cwii"][:], cr_, gt2[:], AO.mult, AO.add)
                nc.scalar.mul(Gn[:, sl], Gi[:, sl], -1.0)

            # ---- big persistent tiles ----
            Abig = bp.tile([128, 10240], f32r, tag="Abig", name="Abig")
            Ubig = bp.tile([128, 10240], bf16, tag="Ubig", name="Ubig")
            u2 = bp.tile([120, 24 * 512], bf16, tag="u2", name="u2")

            i2_offs = []
            off = 0
            for g in range(8):
                for j, cnt in enumerate(IBLK_I2):
                    i2_offs.append((g, j, cnt, off))
                    off += cnt * N2OUT

            # engine-balance counter for I2 magnitude
            alt = [0]

            def emit_f1(c, g, ph, evict_act):
                ag = gp.tile([80, 1024], f32r, tag="ag", name="ag")
                P = psa.tile([80, 1024], f32, tag="pA", name="pF1")
                for jh in range(2):
                    jj = 2 * ph + jh
                    pw = slice(32 * jj, 32 * jj + 32)
                    cwd = slice(80 * g, 80 * (g + 1))
                    rr = xt_r[pw, g * 512 + c * 256 : g * 512 + (c + 1) * 256]
                    ri = xt_i[pw, g * 512 + c * 256 : g * 512 + (c + 1) * 256]
                    lr = ct["cf1r"][pw, cwd]
                    li = ct["cf1i"][pw, cwd]
                    ln = ct["cf1n"][pw, cwd]
                    tpos = (32 * jj, 0)
                    pr_sl = P[:, jh * 512 : jh * 512 + 256]
                    pi_sl = P[:, jh * 512 + 256 : (jh + 1) * 512]
                    nc.tensor.matmul(pr_sl, lr, rr, start=True, stop=False, tile_position=tpos)
                    nc.tensor.matmul(pr_sl, ln, ri, start=False, stop=True, tile_position=tpos)
                    nc.tensor.matmul(pi_sl, li, rr, start=True, stop=False, tile_position=tpos)
                    nc.tensor.matmul(pi_sl, lr, ri, start=False, stop=True, tile_position=tpos)
                nc.scalar.activation(ag[:, 0:512], P[:, 0:512], AF.Copy)
                nc.vector.tensor_copy(ag[:, 512:1024], P[:, 512:1024])
                for jh in range(2):
                    jj = 2 * ph + jh
                    eng = nc.sync if jh == 0 else nc.gpsimd
                    eng.dma_start(
                        out=_ap(Abig, [[10240, 4], [1, 10240]],
                                extra_off=(16 * g + 4 * jj) * 10240),
                        in_=ag[:, jh * 512 : (jh + 1) * 512],
                    )

            zp_hist = {}

            def emit_f3(kp):
                k2a = 2 * kp
                PX = psa.tile([128, 1024], f32, tag="pA", name="pF3")
                for kh in range(2):
                    k2 = k2a + kh
                    asl_r = slice(k2 * 512, k2 * 512 + 256)
                    asl_i = slice(k2 * 512 + 256, (k2 + 1) * 512)
                    pr_sl = PX[:, kh * 512 : kh * 512 + 256]
                    pi_sl = PX[:, kh * 512 + 256 : (kh + 1) * 512]
                    nc.tensor.matmul(pr_sl, ct["cwfr"][:], Abig[:, asl_r], start=True, stop=False)
                    nc.tensor.matmul(pr_sl, ct["cwfn"][:], Abig[:, asl_i], start=False, stop=True)
                    nc.tensor.matmul(pi_sl, ct["cwfi"][:], Abig[:, asl_r], start=True, stop=False)
                    nc.tensor.matmul(pi_sl, ct["cwfr"][:], Abig[:, asl_i], start=False, stop=True)
                xr_view = _ap(PX, [[1024, 128], [512, 2], [1, 256]])
                xi_view = _ap(PX, [[1024, 128], [512, 2], [1, 256]], extra_off=256)
                xr_s = qp.tile([128, 512], bf16, tag="xr_s", name="xr_s")
                m1 = qp.tile([128, 512], bf16, tag="m1", name="m1")
                m2 = qp.tile([128, 512], bf16, tag="m2", name="m2")
                Zrp = zp.tile([128, 512], bf16, tag="Zrp", name="Zrp")
                Ptp = zp.tile([128, 512], bf16, tag="Ptp", name="Ptp")
                nc.scalar.activation(m1[:], xr_view, AF.Square)
                nc.scalar.activation(m2[:], xi_view, AF.Square)
                nc.vector.tensor_copy(xr_s[:], xr_view)
                nc.vector.scalar_tensor_tensor(Ptp[:], xr_s[:], 2.0, xi_view, AO.mult, AO.mult)
                nc.vector.tensor_sub(Zrp[:], m1[:], m2[:])
                zp_hist[kp] = (Zrp, Ptp)

            def emit_i1(kp):
                k2a = 2 * kp
                Zrp, Ptp = zp_hist.pop(kp)
                PU = psa.tile([128, 1024], f32, tag="pB", name="pI1")
                for kh in range(2):
                    k2 = k2a + kh
                    zsl = slice(kh * 256, (kh + 1) * 256)
                    gsl = slice(k2 * 128, (k2 + 1) * 128)
                    pr_sl = PU[:, kh * 512 : kh * 512 + 256]
                    pi_sl = PU[:, kh * 512 + 256 : (kh + 1) * 512]
                    nc.tensor.matmul(pr_sl, Gr[:, gsl], Zrp[:, zsl], start=True, stop=False)
                    nc.tensor.matmul(pr_sl, Gn[:, gsl], Ptp[:, zsl], start=False, stop=True)
                    nc.tensor.matmul(pi_sl, Gi[:, gsl], Zrp[:, zsl], start=True, stop=False)
                    nc.tensor.matmul(pi_sl, Gr[:, gsl], Ptp[:, zsl], start=False, stop=True)
                dst_lo = Ubig[:, k2a * 512 : (k2a + 1) * 512]
                dst_hi = Ubig[:, (k2a + 1) * 512 : (k2a + 2) * 512]
                nc.scalar.activation(dst_lo, PU[:, 0:512], AF.Copy)
                nc.vector.tensor_copy(dst_hi, PU[:, 512:1024])

            def emit_pivd(c):
                for idx, (g, j, cnt, off) in enumerate(i2_offs):
                    n1_0 = 16 * g + JOFS_I2[j]
                    eng = nc.sync if idx % 2 == 0 else nc.gpsimd
                    eng.dma_start(
                        out=_ap(u2, [[24 * 512, cnt * 20], [1, 512]], extra_off=idx * 512),
                        in_=_ap(Ubig, [[10240, cnt], [1, 10240]], extra_off=n1_0 * 10240),
                    )

            def emit_i2(c, g):
                idx0 = 3 * g
                off0 = i2_offs[idx0][3]
                PY = psa.tile([102, 1024], f32, tag="pB", name="pI2p")
                for jh in range(2):
                    idx = idx0 + jh
                    (_, _, cnt, off) = i2_offs[idx]
                    Kj, Mj = cnt * 20, cnt * N2OUT
                    csl = slice(off, off + Mj)
                    usl_r = slice(idx * 512, idx * 512 + 256)
                    usl_i = slice(idx * 512 + 256, (idx + 1) * 512)
                    pr_sl = PY[:Mj, jh * 512 : jh * 512 + 256]
                    pi_sl = PY[:Mj, jh * 512 + 256 : (jh + 1) * 512]
                    nc.tensor.matmul(pr_sl, ct["ci2r"][:Kj, csl], u2[:Kj, usl_r], start=True, stop=False)
                    nc.tensor.matmul(pr_sl, ct["ci2n"][:Kj, csl], u2[:Kj, usl_i], start=False, stop=True)
                    nc.tensor.matmul(pi_sl, ct["ci2i"][:Kj, csl], u2[:Kj, usl_r], start=True, stop=False)
                    nc.tensor.matmul(pi_sl, ct["ci2r"][:Kj, csl], u2[:Kj, usl_i], start=False, stop=True)
                yr_view = _ap(PY, [[1024, 102], [512, 2], [1, 256]])
                yi_view = _ap(PY, [[1024, 102], [512, 2], [1, 256]], extra_off=256)
                s1 = sp.tile([102, 512], bf16, tag="s1", name="s1")
                s2 = sp.tile([102, 512], bf16, tag="s2", name="s2")
                ssum = sp.tile([102, 512], bf16, tag="ssum", name="ssum")
                ya_p = sp.tile([102, 512], bf16, tag="ya_p", name="ya_p")
                nc.scalar.activation(s1[:], yr_view, AF.Square)
                nc.scalar.activation(s2[:], yi_view, AF.Square)
                nc.vector.tensor_add(ssum[:], s1[:], s2[:])
                nc.scalar.activation(ya_p[:], ssum[:], AF.Sqrt)
                ysl = yraw[off0 : off0 + 204, c * CHUNK : (c + 1) * CHUNK]
                nc.sync.dma_start(
                    out=bass.AP(ysl.tensor, ysl.offset,
                                [[BCORE, 102], [102 * BCORE, 2], [1, 256]]),
                    in_=_ap(ya_p, [[512, 102], [256, 2], [1, 256]]),
                )
                idx = idx0 + 2
                (_, _, cnt, off) = i2_offs[idx]
                Kj, Mj = cnt * 20, cnt * N2OUT
                csl = slice(off, off + Mj)
                usl_r = slice(idx * 512, idx * 512 + 256)
                usl_i = slice(idx * 512 + 256, (idx + 1) * 512)
                PS = psa.tile([68, 512], f32, tag="pB", name="pI2s")
                nc.tensor.matmul(PS[:Mj, 0:256], ct["ci2r"][:Kj, csl], u2[:Kj, usl_r], start=True, stop=False)
                nc.tensor.matmul(PS[:Mj, 0:256], ct["ci2n"][:Kj, csl], u2[:Kj, usl_i], start=False, stop=True)
                nc.tensor.matmul(PS[:Mj, 256:512], ct["ci2i"][:Kj, csl], u2[:Kj, usl_r], start=True, stop=False)
                nc.tensor.matmul(PS[:Mj, 256:512], ct["ci2r"][:Kj, csl], u2[:Kj, usl_i], start=False, stop=True)
                t1 = sp.tile([68, 256], bf16, tag="t1", name="t1")
                t2 = sp.tile([68, 256], bf16, tag="t2", name="t2")
                ts_ = sp.tile([68, 256], bf16, tag="ts_", name="ts_")
                ya_s = sp.tile([68, 256], bf16, tag="ya_s", name="ya_s")
                nc.scalar.activation(t1[:], PS[:68, 0:256], AF.Square)
                xi2s = sp.tile([68, 256], bf16, tag="xi2s", name="xi2s")
                nc.vector.tensor_copy(xi2s[:], PS[:68, 256:512])
                nc.vector.tensor_mul(t2[:], xi2s[:], xi2s[:])
                nc.vector.tensor_add(ts_[:], t1[:], t2[:])
                nc.scalar.activation(ya_s[:], ts_[:], AF.Sqrt)
                nc.sync.dma_start(
                    out=yraw[off : off + Mj, c * CHUNK : (c + 1) * CHUNK],
                    in_=ya_s[:Mj, :],
                )

            # ================= issue schedule =================
            # chunk 0: F1 (all evicts on ACT; DVE is busy waiting on W-prep)
            for g in range(8):
                for ph in range(2):
                    emit_f1(0, g, ph, evict_act=True)
            # G variants for the first pairs (C becomes ready during F1)
            for k2 in range(6):
                emit_g(k2)
            # chunk 0: F3 + I1 with remaining G interleaved
            for kp in range(11):
                if kp < 7:
                    emit_g(6 + 2 * kp)
                    emit_g(7 + 2 * kp)
                if kp < 10:
                    emit_f3(kp)
                if kp >= 1:
                    emit_i1(kp - 1)
            emit_pivd(0)
            # fused: chunk-0 I2 interleaved with chunk-1 F1
            for g in range(8):
                emit_i2(0, g)
                emit_f1(1, g, 0, evict_act=(g % 2 == 0))
                emit_f1(1, g, 1, evict_act=(g % 2 == 1))
            # chunk 1: F3 + I1
            for kp in range(11):
                if kp < 10:
                    emit_f3(kp)
                if kp >= 1:
                    emit_i1(kp - 1)
            emit_pivd(1)
            for g in range(8):
                emit_i2(1, g)

    nc.compile()
    return nc


_NC_CACHE = None


def kernel(**inputs):
    global _NC_CACHE
    x_real = np.ascontiguousarray(inputs["x_real"], dtype=np.float32)
    x_imag = np.ascontiguousarray(inputs["x_imag"], dtype=np.float32)
    w0_real = np.ascontiguousarray(inputs["w0_real"], dtype=np.float32)
    w0_imag = np.ascontiguousarray(inputs["w0_imag"], dtype=np.float32)
    wl_real = np.ascontiguousarray(inputs["wl_real"], dtype=np.float32)
    wl_imag = np.ascontiguousarray(inputs["wl_imag"], dtype=np.float32)

    xp_r = x_real[:, XPERM]
    xp_i = x_imag[:, XPERM]

    const_maps = {nm: np.ascontiguousarray(arr) for nm, arr in CONSTS.items()}
    in_maps = []
    for cid in range(NCORES):
        rows = slice(cid * BCORE, (cid + 1) * BCORE)
        m = {
            "xh_r": np.ascontiguousarray(xp_r[rows].T),
            "xh_i": np.ascontiguousarray(xp_i[rows].T),
            "w0r": w0_real, "w0i": w0_imag,
            "wlr": wl_real, "wli": wl_imag,
        }
        m.update(const_maps)
        in_maps.append(m)

    if _NC_CACHE is None:
        _NC_CACHE = build_nc()
    res = run_bass_kernel_spmd(_NC_CACHE, in_maps, core_ids=list(range(NCORES)))
    global _LAST_RES
    _LAST_RES = res

    out = np.empty((B, CLASS_NUM), np.float32)
    cols = YN[YVALID] - CROP0
    for cid in range(NCORES):
        yr = res.results[cid]["yraw"]  # [2176, 512] bf16
        out[cid * BCORE : (cid + 1) * BCORE, cols] = yr[YVALID].T.astype(np.float32)
    return out


# revision 5
# speedup vs baseline: 1.0431x; 1.0431x over previous
"""Trainium2 Bass kernel for nn_CNN_Comp_29240137351522 (dense_cnn).

Math:  y = |IFFT_N( FFT_N(x)^2 * C )[255:2303]|,  C = FFT_N(w0)^2 * FFT_N(wl) / N
with N = 2560 = 128*20 so the chained full convolutions are exact.

v2 changes vs baseline:
  - host pre-transposes x (feature-major), eliminating on-device transposes
    and their PSUM evictions
  - plane-combined PSUM tiles [*, 1024] = (half, plane, b-256) so each
    PSUM->SBUF eviction is ONE wide engine op instead of two
  - bf16 intermediates from F1 output onward (Abig, Zr/Pt, Ubig, u2) for
    2x DVE math and half-size pivot DMAs
  - square/magnitude math on DVE in bf16 (2x mode), balanced against ACT
  - stores split to Pool SWDGE; loads/pivots on SP HWDGE
"""

import numpy as np
import ml_dtypes

import concourse.bass as bass
import concourse.bacc as bacc
import concourse.mybir as mybir
from concourse.tile import TileContext
from concourse.bass_utils import run_bass_kernel_spmd

# ---------------- static problem config ----------------
B, NX = 4096, 1024
K0, KL = 129, 257
N = 2560
N1, N2 = 128, 20
NCORES = 8
BCORE = B // NCORES          # 512
CHUNK = 256
NCHUNKS = BCORE // CHUNK     # 2
N2OUT = 17                   # n2 in [1,18)
CROP0 = 255
CLASS_NUM = 2048
IBLK_I2 = (6, 6, 4)
JOFS_I2 = (0, 6, 12)
YRAW_ROWS = 8 * sum(IBLK_I2) * N2OUT  # 2176

f32 = mybir.dt.float32
f32r = mybir.dt.float32r
bf16 = mybir.dt.bfloat16
AO = mybir.AluOpType
AF = mybir.ActivationFunctionType


def _w(num, den):
    return np.exp(-2j * np.pi * np.asarray(num, np.float64) / den)


# ---------------- host-side constant arrays ----------------
def _build_consts():
    c = {}
    n1g = np.arange(N1)
    k1g = np.arange(N1)
    k2g = np.arange(N2)
    n2g8 = np.arange(8)

    # F1 lhsT: [128, 640]; block (g,jj) at partitions [32jj,32jj+32), cols [80g,80g+80)
    # rows (il in 4)*8 + n2, cols il*20 + k2; value W20[n2,k2] * W2560^{n1 k2}, n1=16g+4jj+il
    f1 = np.zeros((128, 640), np.complex128)
    for g in range(8):
        for jj in range(4):
            for il in range(4):
                n1 = 16 * g + 4 * jj + il
                blk = _w(np.outer(n2g8, k2g), N2) * _w(n1 * k2g, N)[None, :]
                f1[32 * jj + il * 8 : 32 * jj + il * 8 + 8, 80 * g + il * 20 : 80 * g + (il + 1) * 20] = blk
    c["cf1r"] = f1.real.astype(np.float32)
    c["cf1i"] = f1.imag.astype(np.float32)
    c["cf1n"] = (-f1.imag).astype(np.float32)

    # F3 lhsT (shared, f32r): W128[n1,k1] -- also used by the weight-DFT
    w3 = _w(np.outer(n1g, k1g), N1)
    c["cwfr"] = w3.real.astype(np.float32)
    c["cwfi"] = w3.imag.astype(np.float32)
    c["cwfn"] = (-w3.imag).astype(np.float32)

    # I1 base: W128i[k1,n1] (bf16, G built on device)
    wi = _w(-np.outer(k1g, n1g), N1)
    c["cwir"] = wi.real.astype(ml_dtypes.bfloat16)
    c["cwii"] = wi.imag.astype(ml_dtypes.bfloat16)

    # I2 lhsT: [120, 2176]; per (g,j) cols [off,off+M_j); block-diag il:
    # rows il*20+k2, cols il*17+(n2-1); value W20^{-k2 n2} * W2560^{-n1 k2}
    n2out = np.arange(1, 18)
    i2 = np.zeros((120, 2176), np.complex128)
    off = 0
    for g in range(8):
        for j, cnt in enumerate(IBLK_I2):
            for il in range(cnt):
                n1 = 16 * g + JOFS_I2[j] + il
                blk = _w(-np.outer(k2g, n2out), N2) * _w(-n1 * k2g, N)[:, None]
                i2[il * 20 : (il + 1) * 20, off + il * 17 : off + (il + 1) * 17] = blk
            off += cnt * N2OUT
    c["ci2r"] = i2.real.astype(ml_dtypes.bfloat16)
    c["ci2i"] = i2.imag.astype(ml_dtypes.bfloat16)
    c["ci2n"] = (-i2.imag).astype(ml_dtypes.bfloat16)

    # weight-DFT rhs constants
    nh = np.arange(128)
    t129 = _w(np.outer(nh, k2g), N)
    c["ct1r"] = t129.real.astype(np.float32)
    c["ct1i"] = t129.imag.astype(np.float32)
    t257b = _w(np.outer(nh, k2g), N) * _w(k2g, 20)[None, :]
    c["ct2r"] = t257b.real.astype(np.float32)
    c["ct2i"] = t257b.imag.astype(np.float32)
    t129e = _w(k2g, 20)
    c["te1r"] = t129e.real.astype(np.float32).reshape(1, N2)
    c["te1i"] = t129e.imag.astype(np.float32).reshape(1, N2)
    t257e = _w(k2g, 10)
    c["te2r"] = t257e.real.astype(np.float32).reshape(1, N2)
    c["te2i"] = t257e.imag.astype(np.float32).reshape(1, N2)

    c["ones1"] = np.ones((1, 128), np.float32)
    return c


CONSTS = _build_consts()


def host_x_perm():
    """perm[g*128 + i*8 + n2] = n2*128 + 16g + i"""
    perm = np.empty(NX, np.int64)
    for g in range(8):
        for i in range(16):
            for n2 in range(8):
                perm[g * 128 + i * 8 + n2] = n2 * 128 + 16 * g + i
    return perm


def yraw_maps():
    """row r of yraw -> output column (n-255), valid mask."""
    rows = []
    for g in range(8):
        for j, cnt in enumerate(IBLK_I2):
            for il in range(cnt):
                n1 = 16 * g + JOFS_I2[j] + il
                for q in range(N2OUT):
                    rows.append((q + 1) * 128 + n1)
    narr = np.array(rows)
    valid = (narr >= CROP0) & (narr < CROP0 + CLASS_NUM)
    return narr, valid


XPERM = host_x_perm()
YN, YVALID = yraw_maps()


def _ap(tile, dims, extra_off=0):
    return bass.AP(tile.tensor, tile[:].offset + extra_off, dims)


DEBUG_TAPS = False
AG_DT = bf16  # flip to f32r to debug precision/corruption
XP_BUFS = 2
GP_BUFS = 2
SP_BUFS = 2
EVICT_MODE = "wide-alt"
GSCALE_DVE = False


# ---------------- bass kernel builder ----------------
def build_nc():
    nc = bacc.Bacc("TRN2", target_bir_lowering=False, debug=False, num_devices=NCORES)

    # DRAM tensors: xh = pre-transposed permuted x, [NX, BCORE]
    d = {}
    d["xh_r"] = nc.dram_tensor("xh_r", [NX, BCORE], f32r, kind="ExternalInput")
    d["xh_i"] = nc.dram_tensor("xh_i", [NX, BCORE], f32r, kind="ExternalInput")
    for nm, shape in [("w0r", [K0]), ("w0i", [K0]), ("wlr", [KL]), ("wli", [KL])]:
        d[nm] = nc.dram_tensor(nm, shape, f32, kind="ExternalInput")
    cdt = {"cf1r": f32r, "cf1i": f32r, "cf1n": f32r,
           "cwfr": f32r, "cwfi": f32r, "cwfn": f32r,
           "cwir": bf16, "cwii": bf16,
           "ci2r": bf16, "ci2i": bf16, "ci2n": bf16,
           "ones1": f32r}
    for nm, arr in CONSTS.items():
        d[nm] = nc.dram_tensor(nm, list(arr.shape), cdt.get(nm, f32), kind="ExternalInput")
    yraw = nc.dram_tensor("yraw", [YRAW_ROWS, BCORE], bf16, kind="ExternalOutput")
    dbg = {}
    if DEBUG_TAPS:
        for nm, shape, ddt in [("dag", [80, 2048], AG_DT), ("dagf", [80, 2048], f32), ("dAbig", [128, 10240], AG_DT), ("dZr", [128, 5120], bf16),
                               ("dPt", [128, 5120], bf16), ("dUbig", [128, 10240], bf16),
                               ("du2", [120, 24 * 512], bf16)]:
            dbg[nm] = nc.dram_tensor(nm, shape, ddt, kind="ExternalOutput")

    with TileContext(nc) as tc:
        with (
            tc.tile_pool(name="cp", bufs=1) as cp,         # consts + persistent
            tc.tile_pool(name="bp", bufs=1) as bp,         # big single-buffer tiles
            tc.tile_pool(name="xp", bufs=1) as xp,         # chunk input tiles
            tc.tile_pool(name="gp", bufs=GP_BUFS) as gp,
            tc.tile_pool(name="zp", bufs=4) as zp,
            tc.tile_pool(name="qp", bufs=3) as qp,         # ag staging
            tc.tile_pool(name="sp", bufs=SP_BUFS) as sp,         # small rotating scratch
            tc.tile_pool(name="tp", bufs=3) as tp,         # f32 tmp tiles (weight prep)
            tc.tile_pool(name="psa", bufs=2, space="PSUM") as psa,  # 2 tags x 2 bufs x 2 banks
        ):
            # ---- input loads (halves) interleaved with critical consts ----
            xt_r = xp.tile([128, 4096], f32r, tag="xtr", name="xt_r")
            xt_i = xp.tile([128, 4096], f32r, tag="xti", name="xt_i")
            def load_quarter(q):
                for xt, srcnm in [(xt_r, "xh_r"), (xt_i, "xh_i")]:
                    sap = d[srcnm][:, :]
                    nc.sync.dma_start(
                        out=_ap(xt, [[4096, 128], [512, 2], [1, 512]],
                                extra_off=q * 1024),
                        in_=bass.AP(sap.tensor, sap.offset + q * 2 * 128 * BCORE,
                                    [[BCORE, 128], [128 * BCORE, 2], [1, 512]]),
                    )
            def load_half(h):
                load_quarter(2 * h)
                load_quarter(2 * h + 1)
            load_half(0)

            ct = {}
            def load_consts(names, eng):
                for nm in names:
                    arr = CONSTS[nm]
                    t = cp.tile(list(arr.shape), cdt.get(nm, f32), tag=nm, name=nm)
                    eng.dma_start(out=t[:], in_=d[nm][:, :] if arr.ndim == 2 else d[nm][:])
                    ct[nm] = t
            load_consts(["cf1r", "cf1i", "cf1n"], nc.sync)
            load_consts(["ct1r", "ct1i", "ct2r", "ct2i",
                         "te1r", "te1i", "te2r", "te2i", "ones1"], nc.gpsimd)
            load_consts(["cwfr", "cwfi", "cwfn", "cwir", "cwii",
                         "ci2r", "ci2i", "ci2n"], nc.gpsimd)

            # ---- load w0/wl pieces as [128,1] / [1,1] columns ----
            wc = {}
            for nm, src, lo, hi in [
                ("w0r_c", "w0r", 0, 128), ("w0i_c", "w0i", 0, 128),
                ("wlr_c1", "wlr", 0, 128), ("wli_c1", "wli", 0, 128),
                ("wlr_c2", "wlr", 128, 256), ("wli_c2", "wli", 128, 256),
            ]:
                t = cp.tile([128, 1], f32, tag=nm, name=nm)
                nc.sync.dma_start(out=t[:], in_=d[src][lo:hi])
                wc[nm] = t
            for nm, src, pos in [("w0r_e", "w0r", 128), ("w0i_e", "w0i", 128),
                                 ("wlr_e", "wlr", 256), ("wli_e", "wli", 256)]:
                t = cp.tile([1, 1], f32, tag=nm, name=nm)
                nc.sync.dma_start(out=t[:], in_=d[src][pos:pos + 1])
                wc[nm] = t
            load_half(1)

            # ---- weight DFT: W0, WL [128, 20] (f32 path, tiny) ----
            def build_rhs(tr, ti, cr_, ci_, out_r, out_i):
                tmp = tp.tile([tr.shape[0], N2], f32, tag="wtmp", name="wtmp")
                nc.vector.tensor_scalar(tmp[:], ti[:], ci_[:], None, AO.mult)
                nc.vector.scalar_tensor_tensor(out_r[:], tr[:], cr_[:], tmp[:], AO.mult, AO.subtract)
                tmp2 = tp.tile([tr.shape[0], N2], f32, tag="wtmp2", name="wtmp2")
                nc.vector.tensor_scalar(tmp2[:], tr[:], ci_[:], None, AO.mult)
                nc.vector.scalar_tensor_tensor(out_i[:], ti[:], cr_[:], tmp2[:], AO.mult, AO.add)

            def weight_dft(chunks, tail, out_r, out_i):
                ps_r = psa.tile([128, N2], f32, tag="pB", name="wpsr")
                ps_i = psa.tile([128, N2], f32, tag="pB", name="wpsi")
                rhs = []
                for (t_r, t_i, colr, coli) in chunks:
                    rr = tp.tile([128, N2], f32r, tag="wrhs_r", name="wrhs_r")
                    ri = tp.tile([128, N2], f32r, tag="wrhs_i", name="wrhs_i")
                    build_rhs(t_r, t_i, colr, coli, rr, ri)
                    rhs.append((rr, ri))
                te_r, te_i, er, ei = tail
                tr = tp.tile([1, N2], f32r, tag="wtail_r", name="wtail_r")
                ti_ = tp.tile([1, N2], f32r, tag="wtail_i", name="wtail_i")
                tmp = tp.tile([1, N2], f32, tag="wtmp3", name="wtmp3")
                nc.vector.tensor_scalar(tmp[:], te_i[:], ei[:], None, AO.mult)
                nc.vector.scalar_tensor_tensor(tr[:], te_r[:], er[:], tmp[:], AO.mult, AO.subtract)
                tmp2 = tp.tile([1, N2], f32, tag="wtmp4", name="wtmp4")
                nc.vector.tensor_scalar(tmp2[:], te_r[:], ei[:], None, AO.mult)
                nc.vector.scalar_tensor_tensor(ti_[:], te_i[:], er[:], tmp2[:], AO.mult, AO.add)
                first = True
                for (rr, ri) in rhs:
                    nc.tensor.matmul(ps_r[:], ct["cwfr"][:], rr[:], start=first, stop=False)
                    nc.tensor.matmul(ps_r[:], ct["cwfn"][:], ri[:], start=False, stop=False)
                    first = False
                nc.tensor.matmul(ps_r[:], ct["ones1"][:1, :], tr[:], start=False, stop=True)
                first = True
                for (rr, ri) in rhs:
                    nc.tensor.matmul(ps_i[:], ct["cwfi"][:], rr[:], start=first, stop=False)
                    nc.tensor.matmul(ps_i[:], ct["cwfr"][:], ri[:], start=False, stop=False)
                    first = False
                nc.tensor.matmul(ps_i[:], ct["ones1"][:1, :], ti_[:], start=False, stop=True)
                nc.vector.tensor_copy(out_r[:], ps_r[:])
                nc.vector.tensor_copy(out_i[:], ps_i[:])

            W0r = cp.tile([128, N2], f32, tag="W0r", name="W0r")
            W0i = cp.tile([128, N2], f32, tag="W0i", name="W0i")
            weight_dft(
                [(ct["ct1r"], ct["ct1i"], wc["w0r_c"], wc["w0i_c"])],
                (ct["te1r"], ct["te1i"], wc["w0r_e"], wc["w0i_e"]),
                W0r, W0i,
            )
            WLr = cp.tile([128, N2], f32, tag="WLr", name="WLr")
            WLi = cp.tile([128, N2], f32, tag="WLi", name="WLi")
            weight_dft(
                [(ct["ct1r"], ct["ct1i"], wc["wlr_c1"], wc["wli_c1"]),
                 (ct["ct2r"], ct["ct2i"], wc["wlr_c2"], wc["wli_c2"])],
                (ct["te2r"], ct["te2i"], wc["wlr_e"], wc["wli_e"]),
                WLr, WLi,
            )

            # ---- C = W0^2 * WL / N  [128, 20] ----
            Cr = cp.tile([128, N2], f32, tag="Cr", name="Cr")
            Ci = cp.tile([128, N2], f32, tag="Ci", name="Ci")
            ta = tp.tile([128, N2], f32, tag="ca", name="ca")
            tb = tp.tile([128, N2], f32, tag="cb", name="cb")
            tm1 = tp.tile([128, N2], f32, tag="cm1", name="cm1")
            tm2 = tp.tile([128, N2], f32, tag="cm2", name="cm2")
            nc.vector.tensor_mul(tm1[:], W0r[:], W0r[:])
            nc.vector.tensor_mul(tm2[:], W0i[:], W0i[:])
            nc.vector.tensor_sub(ta[:], tm1[:], tm2[:])
            nc.vector.tensor_mul(tm1[:], W0r[:], W0i[:])
            nc.vector.tensor_add(tb[:], tm1[:], tm1[:])
            nc.vector.tensor_mul(tm1[:], ta[:], WLr[:])
            nc.vector.tensor_mul(tm2[:], tb[:], WLi[:])
            nc.vector.tensor_sub(tm1[:], tm1[:], tm2[:])
            nc.vector.tensor_scalar(Cr[:], tm1[:], 1.0 / N, None, AO.mult)
            nc.vector.tensor_mul(tm1[:], ta[:], WLi[:])
            nc.vector.tensor_mul(tm2[:], tb[:], WLr[:])
            nc.vector.tensor_add(tm1[:], tm1[:], tm2[:])
            nc.vector.tensor_scalar(Ci[:], tm1[:], 1.0 / N, None, AO.mult)

            # ---- G variants (bf16): G_k2 = C[:,k2] row-scaled W128i ----
            Gr = cp.tile([128, N2 * 128], bf16, tag="Gr", name="Gr")
            Gi = cp.tile([128, N2 * 128], bf16, tag="Gi", name="Gi")
            Gn = cp.tile([128, N2 * 128], bf16, tag="Gn", name="Gn")   # -Gi
            def emit_g(k2):
                """Build G variants for one k2; alternates ts ops ACT/DVE."""
                cr_ = Cr[:, k2 : k2 + 1]
                ci_ = Ci[:, k2 : k2 + 1]
                sl = slice(k2 * 128, (k2 + 1) * 128)
                gt = tp.tile([128, 128], bf16, tag="gtmp", name="gtmp")
                gt2 = tp.tile([128, 128], bf16, tag="gtmp2", name="gtmp2")
                if k2 % 2 == 0:
                    nc.scalar.activation(gt[:], ct["cwii"][:], AF.Copy, scale=ci_)
                    nc.scalar.activation(gt2[:], ct["cwir"][:], AF.Copy, scale=ci_)
                else:
                    nc.vector.tensor_scalar(gt[:], ct["cwii"][:], ci_, None, AO.mult)
                    nc.vector.tensor_scalar(gt2[:], ct["cwir"][:], ci_, None, AO.mult)
                nc.vector.scalar_tensor_tensor(Gr[:, sl], ct["cwir"][:], cr_, gt[:], AO.mult, AO.subtract)
                nc.vector.scalar_tensor_tensor(Gi[:, sl], ct["cwii"][:], cr_, gt2[:], AO.mult, AO.add)
                nc.scalar.mul(Gn[:, sl], Gi[:, sl], -1.0)

            # ---- big persistent tiles ----
            Abig = bp.tile([128, 10240], f32r, tag="Abig", name="Abig")
            Ubig = bp.tile([128, 10240], bf16, tag="Ubig", name="Ubig")
            u2 = bp.tile([120, 24 * 512], bf16, tag="u2", name="u2")

            i2_offs = []
            off = 0
            for g in range(8):
                for j, cnt in enumerate(IBLK_I2):
                    i2_offs.append((g, j, cnt, off))
                    off += cnt * N2OUT

            # engine-balance counter for I2 magnitude
            alt = [0]

            def emit_f1(c, g, ph, evict_act):
                ag = gp.tile([80, 1024], f32r, tag="ag", name="ag")
                P = psa.tile([80, 1024], f32, tag="pA", name="pF1")
                for jh in range(2):
                    jj = 2 * ph + jh
                    pw = slice(32 * jj, 32 * jj + 32)
                    cwd = slice(80 * g, 80 * (g + 1))
                    rr = xt_r[pw, g * 512 + c * 256 : g * 512 + (c + 1) * 256]
                    ri = xt_i[pw, g * 512 + c * 256 : g * 512 + (c + 1) * 256]
                    lr = ct["cf1r"][pw, cwd]
                    li = ct["cf1i"][pw, cwd]
                    ln = ct["cf1n"][pw, cwd]
                    tpos = (32 * jj, 0)
                    pr_sl = P[:, jh * 512 : jh * 512 + 256]
                    pi_sl = P[:, jh * 512 + 256 : (jh + 1) * 512]
                    nc.tensor.matmul(pr_sl, lr, rr, start=True, stop=False, tile_position=tpos)
                    nc.tensor.matmul(pr_sl, ln, ri, start=False, stop=True, tile_position=tpos)
                    nc.tensor.matmul(pi_sl, li, rr, start=True, stop=False, tile_position=tpos)
                    nc.tensor.matmul(pi_sl, lr, ri, start=False, stop=True, tile_position=tpos)
                nc.scalar.activation(ag[:, 0:512], P[:, 0:512], AF.Copy)
                nc.vector.tensor_copy(ag[:, 512:1024], P[:, 512:1024])
                for jh in range(2):
                    jj = 2 * ph + jh
                    eng = nc.sync if jh == 0 else nc.gpsimd
                    eng.dma_start(
                        out=_ap(Abig, [[10240, 4], [1, 10240]],
                                extra_off=(16 * g + 4 * jj) * 10240),
                        in_=ag[:, jh * 512 : (jh + 1) * 512],
                    )

            zp_hist = {}

            def emit_f3(kp):
                k2a = 2 * kp
                PX = psa.tile([128, 1024], f32, tag="pA", name="pF3")
                for kh in range(2):
                    k2 = k2a + kh
                    asl_r = slice(k2 * 512, k2 * 512 + 256)
                    asl_i = slice(k2 * 512 + 256, (k2 + 1) * 512)
                    pr_sl = PX[:, kh * 512 : kh * 512 + 256]
                    pi_sl = PX[:, kh * 512 + 256 : (kh + 1) * 512]
                    nc.tensor.matmul(pr_sl, ct["cwfr"][:], Abig[:, asl_r], start=True, stop=False)
                    nc.tensor.matmul(pr_sl, ct["cwfn"][:], Abig[:, asl_i], start=False, stop=True)
                    nc.tensor.matmul(pi_sl, ct["cwfi"][:], Abig[:, asl_r], start=True, stop=False)
                    nc.tensor.matmul(pi_sl, ct["cwfr"][:], Abig[:, asl_i], start=False, stop=True)
                xr_view = _ap(PX, [[1024, 128], [512, 2], [1, 256]])
                xi_view = _ap(PX, [[1024, 128], [512, 2], [1, 256]], extra_off=256)
                xr_s = qp.tile([128, 512], bf16, tag="xr_s", name="xr_s")
                m1 = qp.tile([128, 512], bf16, tag="m1", name="m1")
                m2 = qp.tile([128, 512], bf16, tag="m2", name="m2")
                Zrp = zp.tile([128, 512], bf16, tag="Zrp", name="Zrp")
                Ptp = zp.tile([128, 512], bf16, tag="Ptp", name="Ptp")
                nc.scalar.activation(m1[:], xr_view, AF.Square)
                nc.scalar.activation(m2[:], xi_view, AF.Square)
                nc.vector.tensor_copy(xr_s[:], xr_view)
                nc.vector.scalar_tensor_tensor(Ptp[:], xr_s[:], 2.0, xi_view, AO.mult, AO.mult)
                nc.vector.tensor_sub(Zrp[:], m1[:], m2[:])
                zp_hist[kp] = (Zrp, Ptp)

            def emit_i1(kp):
                k2a = 2 * kp
                Zrp, Ptp = zp_hist.pop(kp)
                PU = psa.tile([128, 1024], f32, tag="pB", name="pI1")
                for kh in range(2):
                    k2 = k2a + kh
                    zsl = slice(kh * 256, (kh + 1) * 256)
                    gsl = slice(k2 * 128, (k2 + 1) * 128)
                    pr_sl = PU[:, kh * 512 : kh * 512 + 256]
                    pi_sl = PU[:, kh * 512 + 256 : (kh + 1) * 512]
                    nc.tensor.matmul(pr_sl, Gr[:, gsl], Zrp[:, zsl], start=True, stop=False)
                    nc.tensor.matmul(pr_sl, Gn[:, gsl], Ptp[:, zsl], start=False, stop=True)
                    nc.tensor.matmul(pi_sl, Gi[:, gsl], Zrp[:, zsl], start=True, stop=False)
                    nc.tensor.matmul(pi_sl, Gr[:, gsl], Ptp[:, zsl], start=False, stop=True)
                dst_lo = Ubig[:, k2a * 512 : (k2a + 1) * 512]
                dst_hi = Ubig[:, (k2a + 1) * 512 : (k2a + 2) * 512]
                nc.scalar.activation(dst_lo, PU[:, 0:512], AF.Copy)
                nc.vector.tensor_copy(dst_hi, PU[:, 512:1024])

            def emit_pivd(c):
                for idx, (g, j, cnt, off) in enumerate(i2_offs):
                    n1_0 = 16 * g + JOFS_I2[j]
                    eng = nc.sync if idx % 2 == 0 else nc.gpsimd
                    eng.dma_start(
                        out=_ap(u2, [[24 * 512, cnt * 20], [1, 512]], extra_off=idx * 512),
                        in_=_ap(Ubig, [[10240, cnt], [1, 10240]], extra_off=n1_0 * 10240),
                    )

            def emit_i2(c, g):
                idx0 = 3 * g
                off0 = i2_offs[idx0][3]
                PY = psa.tile([102, 1024], f32, tag="pB", name="pI2p")
                for jh in range(2):
                    idx = idx0 + jh
                    (_, _, cnt, off) = i2_offs[idx]
                    Kj, Mj = cnt * 20, cnt * N2OUT
                    csl = slice(off, off + Mj)
                    usl_r = slice(idx * 512, idx * 512 + 256)
                    usl_i = slice(idx * 512 + 256, (idx + 1) * 512)
                    pr_sl = PY[:Mj, jh * 512 : jh * 512 + 256]
                    pi_sl = PY[:Mj, jh * 512 + 256 : (jh + 1) * 512]
                    nc.tensor.matmul(pr_sl, ct["ci2r"][:Kj, csl], u2[:Kj, usl_r], start=True, stop=False)
                    nc.tensor.matmul(pr_sl, ct["ci2n"][:Kj, csl], u2[:Kj, usl_i], start=False, stop=True)
                    nc.tensor.matmul(pi_sl, ct["ci2i"][:Kj, csl], u2[:Kj, usl_r], start=True, stop=False)
                    nc.tensor.matmul(pi_sl, ct["ci2r"][:Kj, csl], u2[:Kj, usl_i], start=False, stop=True)
                yr_view = _ap(PY, [[1024, 102], [512, 2], [1, 256]])
                yi_view = _ap(PY, [[1024, 102], [512, 2], [1, 256]], extra_off=256)
                s1 = sp.tile([102, 512], bf16, tag="s1", name="s1")
                s2 = sp.tile([102, 512], bf16, tag="s2", name="s2")
                ssum = sp.tile([102, 512], bf16, tag="ssum", name="ssum")
                ya_p = sp.tile([102, 512], bf16, tag="ya_p", name="ya_p")
                nc.scalar.activation(s1[:], yr_view, AF.Square)
                nc.scalar.activation(s2[:], yi_view, AF.Square)
                nc.vector.tensor_add(ssum[:], s1[:], s2[:])
                nc.scalar.activation(ya_p[:], ssum[:], AF.Sqrt)
                ysl = yraw[off0 : off0 + 204, c * CHUNK : (c + 1) * CHUNK]
                nc.sync.dma_start(
                    out=bass.AP(ysl.tensor, ysl.offset,
                                [[BCORE, 102], [102 * BCORE, 2], [1, 256]]),
                    in_=_ap(ya_p, [[512, 102], [256, 2], [1, 256]]),
                )
                idx = idx0 + 2
                (_, _, cnt, off) = i2_offs[idx]
                Kj, Mj = cnt * 20, cnt * N2OUT
                csl = slice(off, off + Mj)
                usl_r = slice(idx * 512, idx * 512 + 256)
                usl_i = slice(idx * 512 + 256, (idx + 1) * 512)
                PS = psa.tile([68, 512], f32, tag="pB", name="pI2s")
                nc.tensor.matmul(PS[:Mj, 0:256], ct["ci2r"][:Kj, csl], u2[:Kj, usl_r], start=True, stop=False)
                nc.tensor.matmul(PS[:Mj, 0:256], ct["ci2n"][:Kj, csl], u2[:Kj, usl_i], start=False, stop=True)
                nc.tensor.matmul(PS[:Mj, 256:512], ct["ci2i"][:Kj, csl], u2[:Kj, usl_r], start=True, stop=False)
                nc.tensor.matmul(PS[:Mj, 256:512], ct["ci2r"][:Kj, csl], u2[:Kj, usl_i], start=False, stop=True)
                t1 = sp.tile([68, 256], bf16, tag="t1", name="t1")
                t2 = sp.tile([68, 256], bf16, tag="t2", name="t2")
                ts_ = sp.tile([68, 256], bf16, tag="ts_", name="ts_")
                ya_s = sp.tile([68, 256], bf16, tag="ya_s", name="ya_s")
                nc.scalar.activation(t1[:], PS[:68, 0:256], AF.Square)
                xi2s = sp.tile([68, 256], bf16, tag="xi2s", name="xi2s")
                nc.vector.tensor_copy(xi2s[:], PS[:68, 256:512])
                nc.vector.tensor_mul(t2[:], xi2s[:], xi2s[:])
                nc.vector.tensor_add(ts_[:], t1[:], t2[:])
                nc.scalar.activation(ya_s[:], ts_[:], AF.Sqrt)
                nc.sync.dma_start(
                    out=yraw[off : off + Mj, c * CHUNK : (c + 1) * CHUNK],
                    in_=ya_s[:Mj, :],
                )

            # ================= issue schedule =================
            # chunk 0: F1 (all evicts on ACT; DVE is busy waiting on W-prep)
            for g in range(8):
                for ph in range(2):
                    emit_f1(0, g, ph, evict_act=True)
            # G variants for the first pairs (C becomes ready during F1)
            for k2 in range(6):
                emit_g(k2)
            # chunk 0: F3 + I1 with remaining G interleaved
            for kp in range(11):
                if kp < 7:
                    emit_g(6 + 2 * kp)
                    emit_g(7 + 2 * kp)
                if kp < 10:
                    emit_f3(kp)
                if kp >= 1:
                    emit_i1(kp - 1)
            emit_pivd(0)
            # fused: chunk-0 I2 interleaved with chunk-1 F1
            for g in range(8):
                emit_i2(0, g)
                emit_f1(1, g, 0, evict_act=(g % 2 == 0))
                emit_f1(1, g, 1, evict_act=(g % 2 == 1))
            # chunk 1: F3 + I1
            for kp in range(11):
                if kp < 10:
                    emit_f3(kp)
                if kp >= 1:
                    emit_i1(kp - 1)
            emit_pivd(1)
            for g in range(8):
                emit_i2(1, g)

    nc.compile()
    return nc


_NC_CACHE = None


def kernel(**inputs):
    global _NC_CACHE
    x_real = np.ascontiguousarray(inputs["x_real"], dtype=np.float32)
    x_imag = np.ascontiguousarray(inputs["x_imag"], dtype=np.float32)
    w0_real = np.ascontiguousarray(inputs["w0_real"], dtype=np.float32)
    w0_imag = np.ascontiguousarray(inputs["w0_imag"], dtype=np.float32)
    wl_real = np.ascontiguousarray(inputs["wl_real"], dtype=np.float32)
    wl_imag = np.ascontiguousarray(inputs["wl_imag"], dtype=np.float32)

    xp_r = x_real[:, XPERM]
    xp_i = x_imag[:, XPERM]

    const_maps = {nm: np.ascontiguousarray(arr) for nm, arr in CONSTS.items()}
    in_maps = []
    for cid in range(NCORES):
        rows = slice(cid * BCORE, (cid + 1) * BCORE)
        m = {
            "xh_r": np.ascontiguousarray(xp_r[rows].T),
            "xh_i": np.ascontiguousarray(xp_i[rows].T),
            "w0r": w0_real, "w0i": w0_imag,
            "wlr": wl_real, "wli": wl_imag,
        }
        m.update(const_maps)
        in_maps.append(m)

    if _NC_CACHE is None:
        _NC_CACHE = build_nc()
    res = run_bass_kernel_spmd(_NC_CACHE, in_maps, core_ids=list(range(NCORES)))
    global _LAST_RES
    _LAST_RES = res

    out = np.empty((B, CLASS_NUM), np.float32)
    cols = YN[YVALID] - CROP0
    for cid in range(NCORES):
        yr = res.results[cid]["yraw"]  # [2176, 512] bf16
        out[cid * BCORE : (cid + 1) * BCORE, cols] = yr[YVALID].T.astype(np.float32)
    return out


# revision 6
# speedup vs baseline: 1.0587x; 1.0150x over previous
"""Trainium2 Bass kernel for nn_CNN_Comp_29240137351522 (dense_cnn).

Math:  y = |IFFT_N( FFT_N(x)^2 * C )[255:2303]|,  C = FFT_N(w0)^2 * FFT_N(wl) / N
with N = 2560 = 128*20 so the chained full convolutions are exact.

v2 changes vs baseline:
  - host pre-transposes x (feature-major), eliminating on-device transposes
    and their PSUM evictions
  - plane-combined PSUM tiles [*, 1024] = (half, plane, b-256) so each
    PSUM->SBUF eviction is ONE wide engine op instead of two
  - bf16 intermediates from F1 output onward (Abig, Zr/Pt, Ubig, u2) for
    2x DVE math and half-size pivot DMAs
  - square/magnitude math on DVE in bf16 (2x mode), balanced against ACT
  - stores split to Pool SWDGE; loads/pivots on SP HWDGE
"""

import numpy as np
import ml_dtypes

import concourse.bass as bass
import concourse.bacc as bacc
import concourse.mybir as mybir
from concourse.tile import TileContext
from concourse.bass_utils import run_bass_kernel_spmd

# ---------------- static problem config ----------------
B, NX = 4096, 1024
K0, KL = 129, 257
N = 2560
N1, N2 = 128, 20
NCORES = 8
BCORE = B // NCORES          # 512
CHUNK = 256
NCHUNKS = BCORE // CHUNK     # 2
N2OUT = 17                   # n2 in [1,18)
CROP0 = 255
CLASS_NUM = 2048
IBLK_I2 = (6, 6, 4)
JOFS_I2 = (0, 6, 12)
YRAW_ROWS = 8 * sum(IBLK_I2) * N2OUT  # 2176

f32 = mybir.dt.float32
f32r = mybir.dt.float32r
bf16 = mybir.dt.bfloat16
AO = mybir.AluOpType
AF = mybir.ActivationFunctionType


def _w(num, den):
    return np.exp(-2j * np.pi * np.asarray(num, np.float64) / den)


# ---------------- host-side constant arrays ----------------
def _build_consts():
    c = {}
    n1g = np.arange(N1)
    k1g = np.arange(N1)
    k2g = np.arange(N2)
    n2g8 = np.arange(8)

    # F1 lhsT: [128, 640]; block (g,jj) at partitions [32jj,32jj+32), cols [80g,80g+80)
    # rows (il in 4)*8 + n2, cols il*20 + k2; value W20[n2,k2] * W2560^{n1 k2}, n1=16g+4jj+il
    f1 = np.zeros((128, 640), np.complex128)
    for g in range(8):
        for jj in range(4):
            for il in range(4):
                n1 = 16 * g + 4 * jj + il
                blk = _w(np.outer(n2g8, k2g), N2) * _w(n1 * k2g, N)[None, :]
                f1[32 * jj + il * 8 : 32 * jj + il * 8 + 8, 80 * g + il * 20 : 80 * g + (il + 1) * 20] = blk
    c["cf1r"] = f1.real.astype(np.float32)
    c["cf1i"] = f1.imag.astype(np.float32)
    c["cf1n"] = (-f1.imag).astype(np.float32)

    # F3 lhsT (shared, f32r): W128[n1,k1] -- also used by the weight-DFT
    w3 = _w(np.outer(n1g, k1g), N1)
    c["cwfr"] = w3.real.astype(np.float32)
    c["cwfi"] = w3.imag.astype(np.float32)
    c["cwfn"] = (-w3.imag).astype(np.float32)

    # I1 base: W128i[k1,n1] (bf16, G built on device)
    wi = _w(-np.outer(k1g, n1g), N1)
    c["cwir"] = wi.real.astype(ml_dtypes.bfloat16)
    c["cwii"] = wi.imag.astype(ml_dtypes.bfloat16)

    # I2 lhsT: [120, 2176]; per (g,j) cols [off,off+M_j); block-diag il:
    # rows il*20+k2, cols il*17+(n2-1); value W20^{-k2 n2} * W2560^{-n1 k2}
    n2out = np.arange(1, 18)
    i2 = np.zeros((120, 2176), np.complex128)
    off = 0
    for g in range(8):
        for j, cnt in enumerate(IBLK_I2):
            for il in range(cnt):
                n1 = 16 * g + JOFS_I2[j] + il
                blk = _w(-np.outer(k2g, n2out), N2) * _w(-n1 * k2g, N)[:, None]
                i2[il * 20 : (il + 1) * 20, off + il * 17 : off + (il + 1) * 17] = blk
            off += cnt * N2OUT
    c["ci2r"] = i2.real.astype(ml_dtypes.bfloat16)
    c["ci2i"] = i2.imag.astype(ml_dtypes.bfloat16)
    c["ci2n"] = (-i2.imag).astype(ml_dtypes.bfloat16)

    # weight-DFT rhs constants
    nh = np.arange(128)
    t129 = _w(np.outer(nh, k2g), N)
    c["ct1r"] = t129.real.astype(np.float32)
    c["ct1i"] = t129.imag.astype(np.float32)
    t257b = _w(np.outer(nh, k2g), N) * _w(k2g, 20)[None, :]
    c["ct2r"] = t257b.real.astype(np.float32)
    c["ct2i"] = t257b.imag.astype(np.float32)
    t129e = _w(k2g, 20)
    c["te1r"] = t129e.real.astype(np.float32).reshape(1, N2)
    c["te1i"] = t129e.imag.astype(np.float32).reshape(1, N2)
    t257e = _w(k2g, 10)
    c["te2r"] = t257e.real.astype(np.float32).reshape(1, N2)
    c["te2i"] = t257e.imag.astype(np.float32).reshape(1, N2)

    c["ones1"] = np.ones((1, 128), np.float32)
    return c


CONSTS = _build_consts()


def host_x_perm():
    """perm[g*128 + i*8 + n2] = n2*128 + 16g + i"""
    perm = np.empty(NX, np.int64)
    for g in range(8):
        for i in range(16):
            for n2 in range(8):
                perm[g * 128 + i * 8 + n2] = n2 * 128 + 16 * g + i
    return perm


def yraw_maps():
    """row r of yraw -> output column (n-255), valid mask."""
    rows = []
    for g in range(8):
        for j, cnt in enumerate(IBLK_I2):
            for il in range(cnt):
                n1 = 16 * g + JOFS_I2[j] + il
                for q in range(N2OUT):
                    rows.append((q + 1) * 128 + n1)
    narr = np.array(rows)
    valid = (narr >= CROP0) & (narr < CROP0 + CLASS_NUM)
    return narr, valid


XPERM = host_x_perm()
YN, YVALID = yraw_maps()


def _ap(tile, dims, extra_off=0):
    return bass.AP(tile.tensor, tile[:].offset + extra_off, dims)


DEBUG_TAPS = False
AG_DT = bf16  # flip to f32r to debug precision/corruption
XP_BUFS = 2
GP_BUFS = 2
SP_BUFS = 2
EVICT_MODE = "wide-alt"
GSCALE_DVE = False


# ---------------- bass kernel builder ----------------
def build_nc():
    nc = bacc.Bacc("TRN2", target_bir_lowering=False, debug=False, num_devices=NCORES)

    # DRAM tensors: xh = pre-transposed permuted x, [NX, BCORE]
    d = {}
    d["xh_r"] = nc.dram_tensor("xh_r", [NX, BCORE], f32r, kind="ExternalInput")
    d["xh_i"] = nc.dram_tensor("xh_i", [NX, BCORE], f32r, kind="ExternalInput")
    for nm, shape in [("w0r", [K0]), ("w0i", [K0]), ("wlr", [KL]), ("wli", [KL])]:
        d[nm] = nc.dram_tensor(nm, shape, f32, kind="ExternalInput")
    cdt = {"cf1r": f32r, "cf1i": f32r, "cf1n": f32r,
           "cwfr": f32r, "cwfi": f32r, "cwfn": f32r,
           "cwir": bf16, "cwii": bf16,
           "ci2r": bf16, "ci2i": bf16, "ci2n": bf16,
           "ones1": f32r}
    for nm, arr in CONSTS.items():
        d[nm] = nc.dram_tensor(nm, list(arr.shape), cdt.get(nm, f32), kind="ExternalInput")
    yraw = nc.dram_tensor("yraw", [YRAW_ROWS, BCORE], bf16, kind="ExternalOutput")
    dbg = {}
    if DEBUG_TAPS:
        for nm, shape, ddt in [("dag", [80, 2048], AG_DT), ("dagf", [80, 2048], f32), ("dAbig", [128, 10240], AG_DT), ("dZr", [128, 5120], bf16),
                               ("dPt", [128, 5120], bf16), ("dUbig", [128, 10240], bf16),
                               ("du2", [120, 24 * 512], bf16)]:
            dbg[nm] = nc.dram_tensor(nm, shape, ddt, kind="ExternalOutput")

    with TileContext(nc) as tc:
        with (
            tc.tile_pool(name="cp", bufs=1) as cp,         # consts + persistent
            tc.tile_pool(name="bp", bufs=1) as bp,         # big single-buffer tiles
            tc.tile_pool(name="xp", bufs=1) as xp,         # chunk input tiles
            tc.tile_pool(name="gp", bufs=GP_BUFS) as gp,
            tc.tile_pool(name="zp", bufs=4) as zp,
            tc.tile_pool(name="qp", bufs=3) as qp,         # ag staging
            tc.tile_pool(name="sp", bufs=SP_BUFS) as sp,         # small rotating scratch
            tc.tile_pool(name="tp", bufs=3) as tp,         # f32 tmp tiles (weight prep)
            tc.tile_pool(name="psa", bufs=2, space="PSUM") as psa,  # 2 tags x 2 bufs x 2 banks
        ):
            # ---- input loads (halves) interleaved with critical consts ----
            xt_r = xp.tile([128, 4096], f32r, tag="xtr", name="xt_r")
            xt_i = xp.tile([128, 4096], f32r, tag="xti", name="xt_i")
            def load_quarter(q):
                for xt, srcnm in [(xt_r, "xh_r"), (xt_i, "xh_i")]:
                    sap = d[srcnm][:, :]
                    nc.sync.dma_start(
                        out=_ap(xt, [[4096, 128], [512, 2], [1, 512]],
                                extra_off=q * 1024),
                        in_=bass.AP(sap.tensor, sap.offset + q * 2 * 128 * BCORE,
                                    [[BCORE, 128], [128 * BCORE, 2], [1, 512]]),
                    )
            def load_half(h):
                load_quarter(2 * h)
                load_quarter(2 * h + 1)
            load_half(0)

            ct = {}
            def load_consts(names, eng):
                for nm in names:
                    arr = CONSTS[nm]
                    t = cp.tile(list(arr.shape), cdt.get(nm, f32), tag=nm, name=nm)
                    eng.dma_start(out=t[:], in_=d[nm][:, :] if arr.ndim == 2 else d[nm][:])
                    ct[nm] = t
            load_consts(["cf1r", "cf1i", "cf1n"], nc.sync)
            load_consts(["ct1r", "ct1i", "ct2r", "ct2i",
                         "te1r", "te1i", "te2r", "te2i", "ones1"], nc.gpsimd)
            load_consts(["cwfr", "cwfi", "cwfn", "cwir", "cwii",
                         "ci2r", "ci2i", "ci2n"], nc.gpsimd)

            # ---- load w0/wl pieces as [128,1] / [1,1] columns ----
            wc = {}
            for nm, src, lo, hi in [
                ("w0r_c", "w0r", 0, 128), ("w0i_c", "w0i", 0, 128),
                ("wlr_c1", "wlr", 0, 128), ("wli_c1", "wli", 0, 128),
                ("wlr_c2", "wlr", 128, 256), ("wli_c2", "wli", 128, 256),
            ]:
                t = cp.tile([128, 1], f32, tag=nm, name=nm)
                nc.sync.dma_start(out=t[:], in_=d[src][lo:hi])
                wc[nm] = t
            for nm, src, pos in [("w0r_e", "w0r", 128), ("w0i_e", "w0i", 128),
                                 ("wlr_e", "wlr", 256), ("wli_e", "wli", 256)]:
                t = cp.tile([1, 1], f32, tag=nm, name=nm)
                nc.sync.dma_start(out=t[:], in_=d[src][pos:pos + 1])
                wc[nm] = t
            load_half(1)

            # ---- weight DFT: W0, WL [128, 20] (f32 path, tiny) ----
            def build_rhs(tr, ti, cr_, ci_, out_r, out_i):
                tmp = tp.tile([tr.shape[0], N2], f32, tag="wtmp", name="wtmp")
                nc.vector.tensor_scalar(tmp[:], ti[:], ci_[:], None, AO.mult)
                nc.vector.scalar_tensor_tensor(out_r[:], tr[:], cr_[:], tmp[:], AO.mult, AO.subtract)
                tmp2 = tp.tile([tr.shape[0], N2], f32, tag="wtmp2", name="wtmp2")
                nc.vector.tensor_scalar(tmp2[:], tr[:], ci_[:], None, AO.mult)
                nc.vector.scalar_tensor_tensor(out_i[:], ti[:], cr_[:], tmp2[:], AO.mult, AO.add)

            def weight_dft(chunks, tail, out_r, out_i):
                ps_r = psa.tile([128, N2], f32, tag="pB", name="wpsr")
                ps_i = psa.tile([128, N2], f32, tag="pB", name="wpsi")
                rhs = []
                for (t_r, t_i, colr, coli) in chunks:
                    rr = tp.tile([128, N2], f32r, tag="wrhs_r", name="wrhs_r")
                    ri = tp.tile([128, N2], f32r, tag="wrhs_i", name="wrhs_i")
                    build_rhs(t_r, t_i, colr, coli, rr, ri)
                    rhs.append((rr, ri))
                te_r, te_i, er, ei = tail
                tr = tp.tile([1, N2], f32r, tag="wtail_r", name="wtail_r")
                ti_ = tp.tile([1, N2], f32r, tag="wtail_i", name="wtail_i")
                tmp = tp.tile([1, N2], f32, tag="wtmp3", name="wtmp3")
                nc.vector.tensor_scalar(tmp[:], te_i[:], ei[:], None, AO.mult)
                nc.vector.scalar_tensor_tensor(tr[:], te_r[:], er[:], tmp[:], AO.mult, AO.subtract)
                tmp2 = tp.tile([1, N2], f32, tag="wtmp4", name="wtmp4")
                nc.vector.tensor_scalar(tmp2[:], te_r[:], ei[:], None, AO.mult)
                nc.vector.scalar_tensor_tensor(ti_[:], te_i[:], er[:], tmp2[:], AO.mult, AO.add)
                first = True
                for (rr, ri) in rhs:
                    nc.tensor.matmul(ps_r[:], ct["cwfr"][:], rr[:], start=first, stop=False)
                    nc.tensor.matmul(ps_r[:], ct["cwfn"][:], ri[:], start=False, stop=False)
                    first = False
                nc.tensor.matmul(ps_r[:], ct["ones1"][:1, :], tr[:], start=False, stop=True)
                first = True
                for (rr, ri) in rhs:
                    nc.tensor.matmul(ps_i[:], ct["cwfi"][:], rr[:], start=first, stop=False)
                    nc.tensor.matmul(ps_i[:], ct["cwfr"][:], ri[:], start=False, stop=False)
                    first = False
                nc.tensor.matmul(ps_i[:], ct["ones1"][:1, :], ti_[:], start=False, stop=True)
                nc.vector.tensor_copy(out_r[:], ps_r[:])
                nc.vector.tensor_copy(out_i[:], ps_i[:])

            W0r = cp.tile([128, N2], f32, tag="W0r", name="W0r")
            W0i = cp.tile([128, N2], f32, tag="W0i", name="W0i")
            weight_dft(
                [(ct["ct1r"], ct["ct1i"], wc["w0r_c"], wc["w0i_c"])],
                (ct["te1r"], ct["te1i"], wc["w0r_e"], wc["w0i_e"]),
                W0r, W0i,
            )
            WLr = cp.tile([128, N2], f32, tag="WLr", name="WLr")
            WLi = cp.tile([128, N2], f32, tag="WLi", name="WLi")
            weight_dft(
                [(ct["ct1r"], ct["ct1i"], wc["wlr_c1"], wc["wli_c1"]),
                 (ct["ct2r"], ct["ct2i"], wc["wlr_c2"], wc["wli_c2"])],
                (ct["te2r"], ct["te2i"], wc["wlr_e"], wc["wli_e"]),
                WLr, WLi,
            )

            # ---- C = W0^2 * WL / N  [128, 20] ----
            Cr = cp.tile([128, N2], f32, tag="Cr", name="Cr")
            Ci = cp.tile([128, N2], f32, tag="Ci", name="Ci")
            ta = tp.tile([128, N2], f32, tag="ca", name="ca")
            tb = tp.tile([128, N2], f32, tag="cb", name="cb")
            tm1 = tp.tile([128, N2], f32, tag="cm1", name="cm1")
            tm2 = tp.tile([128, N2], f32, tag="cm2", name="cm2")
            nc.vector.tensor_mul(tm1[:], W0r[:], W0r[:])
            nc.vector.tensor_mul(tm2[:], W0i[:], W0i[:])
            nc.vector.tensor_sub(ta[:], tm1[:], tm2[:])
            nc.vector.tensor_mul(tm1[:], W0r[:], W0i[:])
            nc.vector.tensor_add(tb[:], tm1[:], tm1[:])
            nc.vector.tensor_mul(tm1[:], ta[:], WLr[:])
            nc.vector.tensor_mul(tm2[:], tb[:], WLi[:])
            nc.vector.tensor_sub(tm1[:], tm1[:], tm2[:])
            nc.vector.tensor_scalar(Cr[:], tm1[:], 1.0 / N, None, AO.mult)
            nc.vector.tensor_mul(tm1[:], ta[:], WLi[:])
            nc.vector.tensor_mul(tm2[:], tb[:], WLr[:])
            nc.vector.tensor_add(tm1[:], tm1[:], tm2[:])
            nc.vector.tensor_scalar(Ci[:], tm1[:], 1.0 / N, None, AO.mult)

            # ---- G variants (bf16): G_k2 = C[:,k2] row-scaled W128i ----
            Gr = cp.tile([128, N2 * 128], bf16, tag="Gr", name="Gr")
            Gi = cp.tile([128, N2 * 128], bf16, tag="Gi", name="Gi")
            Gn = cp.tile([128, N2 * 128], bf16, tag="Gn", name="Gn")   # -Gi
            def emit_g(k2):
                """Build G variants for one k2; alternates ts ops ACT/DVE."""
                cr_ = Cr[:, k2 : k2 + 1]
                ci_ = Ci[:, k2 : k2 + 1]
                sl = slice(k2 * 128, (k2 + 1) * 128)
                gt = tp.tile([128, 128], bf16, tag="gtmp", name="gtmp")
                gt2 = tp.tile([128, 128], bf16, tag="gtmp2", name="gtmp2")
                if k2 % 2 == 0:
                    nc.scalar.activation(gt[:], ct["cwii"][:], AF.Copy, scale=ci_)
                    nc.scalar.activation(gt2[:], ct["cwir"][:], AF.Copy, scale=ci_)
                else:
                    nc.vector.tensor_scalar(gt[:], ct["cwii"][:], ci_, None, AO.mult)
                    nc.vector.tensor_scalar(gt2[:], ct["cwir"][:], ci_, None, AO.mult)
                nc.vector.scalar_tensor_tensor(Gr[:, sl], ct["cwir"][:], cr_, gt[:], AO.mult, AO.subtract)
                nc.vector.scalar_tensor_tensor(Gi[:, sl], ct["cwii"][:], cr_, gt2[:], AO.mult, AO.add)
                nc.scalar.mul(Gn[:, sl], Gi[:, sl], -1.0)

            # ---- big persistent tiles ----
            Abig = bp.tile([128, 10240], f32r, tag="Abig", name="Abig")
            Ubig = bp.tile([128, 10240], bf16, tag="Ubig", name="Ubig")
            u2 = bp.tile([120, 24 * 512], bf16, tag="u2", name="u2")

            i2_offs = []
            off = 0
            for g in range(8):
                for j, cnt in enumerate(IBLK_I2):
                    i2_offs.append((g, j, cnt, off))
                    off += cnt * N2OUT

            # engine-balance counter for I2 magnitude
            alt = [0]

            def emit_f1(c, g, ph, evict_act):
                ag = gp.tile([80, 1024], f32r, tag="ag", name="ag")
                P = psa.tile([80, 1024], f32, tag="pA", name="pF1")
                for jh in range(2):
                    jj = 2 * ph + jh
                    pw = slice(32 * jj, 32 * jj + 32)
                    cwd = slice(80 * g, 80 * (g + 1))
                    rr = xt_r[pw, g * 512 + c * 256 : g * 512 + (c + 1) * 256]
                    ri = xt_i[pw, g * 512 + c * 256 : g * 512 + (c + 1) * 256]
                    lr = ct["cf1r"][pw, cwd]
                    li = ct["cf1i"][pw, cwd]
                    ln = ct["cf1n"][pw, cwd]
                    tpos = (32 * jj, 0)
                    pr_sl = P[:, jh * 512 : jh * 512 + 256]
                    pi_sl = P[:, jh * 512 + 256 : (jh + 1) * 512]
                    nc.tensor.matmul(pr_sl, lr, rr, start=True, stop=False, tile_position=tpos)
                    nc.tensor.matmul(pr_sl, ln, ri, start=False, stop=True, tile_position=tpos)
                    nc.tensor.matmul(pi_sl, li, rr, start=True, stop=False, tile_position=tpos)
                    nc.tensor.matmul(pi_sl, lr, ri, start=False, stop=True, tile_position=tpos)
                nc.scalar.activation(ag[:, 0:512], P[:, 0:512], AF.Copy)
                nc.vector.tensor_copy(ag[:, 512:1024], P[:, 512:1024])
                for jh in range(2):
                    jj = 2 * ph + jh
                    eng = nc.sync if jh == 0 else nc.gpsimd
                    eng.dma_start(
                        out=_ap(Abig, [[10240, 4], [1, 10240]],
                                extra_off=(16 * g + 4 * jj) * 10240),
                        in_=ag[:, jh * 512 : (jh + 1) * 512],
                    )

            zp_hist = {}

            f3i1_alt = [False]

            def emit_f3(kp):
                k2a = 2 * kp
                t = ("pA" if kp % 2 == 0 else "pB") if f3i1_alt[0] else "pA"
                PX = psa.tile([128, 1024], f32, tag=t, name="pF3")
                for kh in range(2):
                    k2 = k2a + kh
                    asl_r = slice(k2 * 512, k2 * 512 + 256)
                    asl_i = slice(k2 * 512 + 256, (k2 + 1) * 512)
                    pr_sl = PX[:, kh * 512 : kh * 512 + 256]
                    pi_sl = PX[:, kh * 512 + 256 : (kh + 1) * 512]
                    nc.tensor.matmul(pr_sl, ct["cwfr"][:], Abig[:, asl_r], start=True, stop=False)
                    nc.tensor.matmul(pr_sl, ct["cwfn"][:], Abig[:, asl_i], start=False, stop=True)
                    nc.tensor.matmul(pi_sl, ct["cwfi"][:], Abig[:, asl_r], start=True, stop=False)
                    nc.tensor.matmul(pi_sl, ct["cwfr"][:], Abig[:, asl_i], start=False, stop=True)
                xr_view = _ap(PX, [[1024, 128], [512, 2], [1, 256]])
                xi_view = _ap(PX, [[1024, 128], [512, 2], [1, 256]], extra_off=256)
                xr_s = qp.tile([128, 512], bf16, tag="xr_s", name="xr_s")
                m1 = qp.tile([128, 512], bf16, tag="m1", name="m1")
                m2 = qp.tile([128, 512], bf16, tag="m2", name="m2")
                Zrp = zp.tile([128, 512], bf16, tag="Zrp", name="Zrp")
                Ptp = zp.tile([128, 512], bf16, tag="Ptp", name="Ptp")
                nc.scalar.activation(m1[:], xr_view, AF.Square)
                nc.scalar.activation(m2[:], xi_view, AF.Square)
                nc.vector.tensor_copy(xr_s[:], xr_view)
                nc.vector.scalar_tensor_tensor(Ptp[:], xr_s[:], 2.0, xi_view, AO.mult, AO.mult)
                nc.vector.tensor_sub(Zrp[:], m1[:], m2[:])
                zp_hist[kp] = (Zrp, Ptp)

            def emit_i1(kp):
                k2a = 2 * kp
                Zrp, Ptp = zp_hist.pop(kp)
                t = ("pB" if kp % 2 == 0 else "pA") if f3i1_alt[0] else "pB"
                PU = psa.tile([128, 1024], f32, tag=t, name="pI1")
                for kh in range(2):
                    k2 = k2a + kh
                    zsl = slice(kh * 256, (kh + 1) * 256)
                    gsl = slice(k2 * 128, (k2 + 1) * 128)
                    pr_sl = PU[:, kh * 512 : kh * 512 + 256]
                    pi_sl = PU[:, kh * 512 + 256 : (kh + 1) * 512]
                    nc.tensor.matmul(pr_sl, Gr[:, gsl], Zrp[:, zsl], start=True, stop=False)
                    nc.tensor.matmul(pr_sl, Gn[:, gsl], Ptp[:, zsl], start=False, stop=True)
                    nc.tensor.matmul(pi_sl, Gi[:, gsl], Zrp[:, zsl], start=True, stop=False)
                    nc.tensor.matmul(pi_sl, Gr[:, gsl], Ptp[:, zsl], start=False, stop=True)
                dst_lo = Ubig[:, k2a * 512 : (k2a + 1) * 512]
                dst_hi = Ubig[:, (k2a + 1) * 512 : (k2a + 2) * 512]
                nc.scalar.activation(dst_lo, PU[:, 0:512], AF.Copy)
                nc.vector.tensor_copy(dst_hi, PU[:, 512:1024])

            def emit_pivd(c):
                for idx, (g, j, cnt, off) in enumerate(i2_offs):
                    n1_0 = 16 * g + JOFS_I2[j]
                    eng = nc.sync if idx % 2 == 0 else nc.gpsimd
                    eng.dma_start(
                        out=_ap(u2, [[24 * 512, cnt * 20], [1, 512]], extra_off=idx * 512),
                        in_=_ap(Ubig, [[10240, cnt], [1, 10240]], extra_off=n1_0 * 10240),
                    )

            def emit_i2(c, g):
                idx0 = 3 * g
                off0 = i2_offs[idx0][3]
                PY = psa.tile([102, 1024], f32, tag="pB" if (c == 0 or g % 2 == 0) else "pA", name="pI2p")
                for jh in range(2):
                    idx = idx0 + jh
                    (_, _, cnt, off) = i2_offs[idx]
                    Kj, Mj = cnt * 20, cnt * N2OUT
                    csl = slice(off, off + Mj)
                    usl_r = slice(idx * 512, idx * 512 + 256)
                    usl_i = slice(idx * 512 + 256, (idx + 1) * 512)
                    pr_sl = PY[:Mj, jh * 512 : jh * 512 + 256]
                    pi_sl = PY[:Mj, jh * 512 + 256 : (jh + 1) * 512]
                    nc.tensor.matmul(pr_sl, ct["ci2r"][:Kj, csl], u2[:Kj, usl_r], start=True, stop=False)
                    nc.tensor.matmul(pr_sl, ct["ci2n"][:Kj, csl], u2[:Kj, usl_i], start=False, stop=True)
                    nc.tensor.matmul(pi_sl, ct["ci2i"][:Kj, csl], u2[:Kj, usl_r], start=True, stop=False)
                    nc.tensor.matmul(pi_sl, ct["ci2r"][:Kj, csl], u2[:Kj, usl_i], start=False, stop=True)
                yr_view = _ap(PY, [[1024, 102], [512, 2], [1, 256]])
                yi_view = _ap(PY, [[1024, 102], [512, 2], [1, 256]], extra_off=256)
                s1 = sp.tile([102, 512], bf16, tag="s1", name="s1")
                s2 = sp.tile([102, 512], bf16, tag="s2", name="s2")
                ssum = sp.tile([102, 512], bf16, tag="ssum", name="ssum")
                ya_p = sp.tile([102, 512], bf16, tag="ya_p", name="ya_p")
                nc.scalar.activation(s1[:], yr_view, AF.Square)
                nc.scalar.activation(s2[:], yi_view, AF.Square)
                nc.vector.tensor_add(ssum[:], s1[:], s2[:])
                nc.scalar.activation(ya_p[:], ssum[:], AF.Sqrt)
                ysl = yraw[off0 : off0 + 204, c * CHUNK : (c + 1) * CHUNK]
                nc.sync.dma_start(
                    out=bass.AP(ysl.tensor, ysl.offset,
                                [[BCORE, 102], [102 * BCORE, 2], [1, 256]]),
                    in_=_ap(ya_p, [[512, 102], [256, 2], [1, 256]]),
                )
                idx = idx0 + 2
                (_, _, cnt, off) = i2_offs[idx]
                Kj, Mj = cnt * 20, cnt * N2OUT
                csl = slice(off, off + Mj)
                usl_r = slice(idx * 512, idx * 512 + 256)
                usl_i = slice(idx * 512 + 256, (idx + 1) * 512)
                PS = psa.tile([68, 512], f32, tag="pB" if c == 0 else ("pA" if g % 2 == 0 else "pB"), name="pI2s")
                nc.tensor.matmul(PS[:Mj, 0:256], ct["ci2r"][:Kj, csl], u2[:Kj, usl_r], start=True, stop=False)
                nc.tensor.matmul(PS[:Mj, 0:256], ct["ci2n"][:Kj, csl], u2[:Kj, usl_i], start=False, stop=True)
                nc.tensor.matmul(PS[:Mj, 256:512], ct["ci2i"][:Kj, csl], u2[:Kj, usl_r], start=True, stop=False)
                nc.tensor.matmul(PS[:Mj, 256:512], ct["ci2r"][:Kj, csl], u2[:Kj, usl_i], start=False, stop=True)
                t1 = sp.tile([68, 256], bf16, tag="t1", name="t1")
                t2 = sp.tile([68, 256], bf16, tag="t2", name="t2")
                ts_ = sp.tile([68, 256], bf16, tag="ts_", name="ts_")
                ya_s = sp.tile([68, 256], bf16, tag="ya_s", name="ya_s")
                nc.scalar.activation(t1[:], PS[:68, 0:256], AF.Square)
                xi2s = sp.tile([68, 256], bf16, tag="xi2s", name="xi2s")
                nc.vector.tensor_copy(xi2s[:], PS[:68, 256:512])
                nc.vector.tensor_mul(t2[:], xi2s[:], xi2s[:])
                nc.vector.tensor_add(ts_[:], t1[:], t2[:])
                nc.scalar.activation(ya_s[:], ts_[:], AF.Sqrt)
                nc.sync.dma_start(
                    out=yraw[off : off + Mj, c * CHUNK : (c + 1) * CHUNK],
                    in_=ya_s[:Mj, :],
                )

            # ================= issue schedule =================
            # chunk 0: F1 (all evicts on ACT; DVE is busy waiting on W-prep)
            for g in range(8):
                for ph in range(2):
                    emit_f1(0, g, ph, evict_act=True)
            # G variants for the first pairs (C becomes ready during F1)
            for k2 in range(6):
                emit_g(k2)
            # chunk 0: F3 + I1 with remaining G interleaved
            for kp in range(11):
                if kp < 7:
                    emit_g(6 + 2 * kp)
                    emit_g(7 + 2 * kp)
                if kp < 10:
                    emit_f3(kp)
                if kp >= 1:
                    emit_i1(kp - 1)
            emit_pivd(0)
            # fused: chunk-0 I2 interleaved with chunk-1 F1
            for g in range(8):
                emit_i2(0, g)
                emit_f1(1, g, 0, evict_act=(g % 2 == 0))
                emit_f1(1, g, 1, evict_act=(g % 2 == 1))
            # chunk 1: F3 + I1 (parity-alternating psum tags)
            f3i1_alt[0] = True
            for kp in range(11):
                if kp < 10:
                    emit_f3(kp)
                if kp >= 1:
                    emit_i1(kp - 1)
            f3i1_alt[0] = False
            emit_pivd(1)
            for g in range(8):
                emit_i2(1, g)

    nc.compile()
    return nc


_NC_CACHE = None


def kernel(**inputs):
    global _NC_CACHE
    x_real = np.ascontiguousarray(inputs["x_real"], dtype=np.float32)
    x_imag = np.ascontiguousarray(inputs["x_imag"], dtype=np.float32)
    w0_real = np.ascontiguousarray(inputs["w0_real"], dtype=np.float32)
    w0_imag = np.ascontiguousarray(inputs["w0_imag"], dtype=np.float32)
    wl_real = np.ascontiguousarray(inputs["wl_real"], dtype=np.float32)
    wl_imag = np.ascontiguousarray(inputs["wl_imag"], dtype=np.float32)

    xp_r = x_real[:, XPERM]
    xp_i = x_imag[:, XPERM]

    const_maps = {nm: np.ascontiguousarray(arr) for nm, arr in CONSTS.items()}
    in_maps = []
    for cid in range(NCORES):
        rows = slice(cid * BCORE, (cid + 1) * BCORE)
        m = {
            "xh_r": np.ascontiguousarray(xp_r[rows].T),
            "xh_i": np.ascontiguousarray(xp_i[rows].T),
            "w0r": w0_real, "w0i": w0_imag,
            "wlr": wl_real, "wli": wl_imag,
        }
        m.update(const_maps)
        in_maps.append(m)

    if _NC_CACHE is None:
        _NC_CACHE = build_nc()
    res = run_bass_kernel_spmd(_NC_CACHE, in_maps, core_ids=list(range(NCORES)))
    global _LAST_RES
    _LAST_RES = res

    out = np.empty((B, CLASS_NUM), np.float32)
    cols = YN[YVALID] - CROP0
    for cid in range(NCORES):
        yr = res.results[cid]["yraw"]  # [2176, 512] bf16
        out[cid * BCORE : (cid + 1) * BCORE, cols] = yr[YVALID].T.astype(np.float32)
    return out


# revision 7
# speedup vs baseline: 1.0642x; 1.0052x over previous
"""Trainium2 Bass kernel for nn_CNN_Comp_29240137351522 (dense_cnn).

Math:  y = |IFFT_N( FFT_N(x)^2 * C )[255:2303]|,  C = FFT_N(w0)^2 * FFT_N(wl) / N
with N = 2560 = 128*20 so the chained full convolutions are exact.

v2 changes vs baseline:
  - host pre-transposes x (feature-major), eliminating on-device transposes
    and their PSUM evictions
  - plane-combined PSUM tiles [*, 1024] = (half, plane, b-256) so each
    PSUM->SBUF eviction is ONE wide engine op instead of two
  - bf16 intermediates from F1 output onward (Abig, Zr/Pt, Ubig, u2) for
    2x DVE math and half-size pivot DMAs
  - square/magnitude math on DVE in bf16 (2x mode), balanced against ACT
  - stores split to Pool SWDGE; loads/pivots on SP HWDGE
"""

import numpy as np
import ml_dtypes

import concourse.bass as bass
import concourse.bacc as bacc
import concourse.mybir as mybir
from concourse.tile import TileContext
from concourse.bass_utils import run_bass_kernel_spmd

# ---------------- static problem config ----------------
B, NX = 4096, 1024
K0, KL = 129, 257
N = 2560
N1, N2 = 128, 20
NCORES = 8
BCORE = B // NCORES          # 512
CHUNK = 256
NCHUNKS = BCORE // CHUNK     # 2
N2OUT = 17                   # n2 in [1,18)
CROP0 = 255
CLASS_NUM = 2048
IBLK_I2 = (6, 6, 4)
JOFS_I2 = (0, 6, 12)
YRAW_ROWS = 8 * sum(IBLK_I2) * N2OUT  # 2176

f32 = mybir.dt.float32
f32r = mybir.dt.float32r
bf16 = mybir.dt.bfloat16
AO = mybir.AluOpType
AF = mybir.ActivationFunctionType


def _w(num, den):
    return np.exp(-2j * np.pi * np.asarray(num, np.float64) / den)


# ---------------- host-side constant arrays ----------------
def _build_consts():
    c = {}
    n1g = np.arange(N1)
    k1g = np.arange(N1)
    k2g = np.arange(N2)
    n2g8 = np.arange(8)

    # F1 lhsT: [128, 640]; block (g,jj) at partitions [32jj,32jj+32), cols [80g,80g+80)
    # rows (il in 4)*8 + n2, cols il*20 + k2; value W20[n2,k2] * W2560^{n1 k2}, n1=16g+4jj+il
    f1 = np.zeros((128, 640), np.complex128)
    for g in range(8):
        for jj in range(4):
            for il in range(4):
                n1 = 16 * g + 4 * jj + il
                blk = _w(np.outer(n2g8, k2g), N2) * _w(n1 * k2g, N)[None, :]
                f1[32 * jj + il * 8 : 32 * jj + il * 8 + 8, 80 * g + il * 20 : 80 * g + (il + 1) * 20] = blk
    c["cf1r"] = f1.real.astype(np.float32)
    c["cf1i"] = f1.imag.astype(np.float32)
    c["cf1n"] = (-f1.imag).astype(np.float32)

    # F3 lhsT (shared, f32r): W128[n1,k1] -- also used by the weight-DFT
    w3 = _w(np.outer(n1g, k1g), N1)
    c["cwfr"] = w3.real.astype(np.float32)
    c["cwfi"] = w3.imag.astype(np.float32)
    c["cwfn"] = (-w3.imag).astype(np.float32)

    # I1 base: W128i[k1,n1] (bf16, G built on device)
    wi = _w(-np.outer(k1g, n1g), N1)
    c["cwir"] = wi.real.astype(ml_dtypes.bfloat16)
    c["cwii"] = wi.imag.astype(ml_dtypes.bfloat16)

    # I2 lhsT: [120, 2176]; per (g,j) cols [off,off+M_j); block-diag il:
    # rows il*20+k2, cols il*17+(n2-1); value W20^{-k2 n2} * W2560^{-n1 k2}
    n2out = np.arange(1, 18)
    i2 = np.zeros((120, 2176), np.complex128)
    off = 0
    for g in range(8):
        for j, cnt in enumerate(IBLK_I2):
            for il in range(cnt):
                n1 = 16 * g + JOFS_I2[j] + il
                blk = _w(-np.outer(k2g, n2out), N2) * _w(-n1 * k2g, N)[:, None]
                i2[il * 20 : (il + 1) * 20, off + il * 17 : off + (il + 1) * 17] = blk
            off += cnt * N2OUT
    c["ci2r"] = i2.real.astype(ml_dtypes.bfloat16)
    c["ci2i"] = i2.imag.astype(ml_dtypes.bfloat16)
    c["ci2n"] = (-i2.imag).astype(ml_dtypes.bfloat16)

    # weight-DFT rhs constants
    nh = np.arange(128)
    t129 = _w(np.outer(nh, k2g), N)
    c["ct1r"] = t129.real.astype(np.float32)
    c["ct1i"] = t129.imag.astype(np.float32)
    t257b = _w(np.outer(nh, k2g), N) * _w(k2g, 20)[None, :]
    c["ct2r"] = t257b.real.astype(np.float32)
    c["ct2i"] = t257b.imag.astype(np.float32)
    t129e = _w(k2g, 20)
    c["te1r"] = t129e.real.astype(np.float32).reshape(1, N2)
    c["te1i"] = t129e.imag.astype(np.float32).reshape(1, N2)
    t257e = _w(k2g, 10)
    c["te2r"] = t257e.real.astype(np.float32).reshape(1, N2)
    c["te2i"] = t257e.imag.astype(np.float32).reshape(1, N2)

    c["ones1"] = np.ones((1, 128), np.float32)
    return c


CONSTS = _build_consts()


def host_x_perm():
    """perm[g*128 + i*8 + n2] = n2*128 + 16g + i"""
    perm = np.empty(NX, np.int64)
    for g in range(8):
        for i in range(16):
            for n2 in range(8):
                perm[g * 128 + i * 8 + n2] = n2 * 128 + 16 * g + i
    return perm


def yraw_maps():
    """row r of yraw -> output column (n-255), valid mask."""
    rows = []
    for g in range(8):
        for j, cnt in enumerate(IBLK_I2):
            for il in range(cnt):
                n1 = 16 * g + JOFS_I2[j] + il
                for q in range(N2OUT):
                    rows.append((q + 1) * 128 + n1)
    narr = np.array(rows)
    valid = (narr >= CROP0) & (narr < CROP0 + CLASS_NUM)
    return narr, valid


XPERM = host_x_perm()
YN, YVALID = yraw_maps()


def _ap(tile, dims, extra_off=0):
    return bass.AP(tile.tensor, tile[:].offset + extra_off, dims)


DEBUG_TAPS = False
AG_DT = bf16  # flip to f32r to debug precision/corruption
XP_BUFS = 2
GP_BUFS = 2
SP_BUFS = 2
EVICT_MODE = "wide-alt"
GSCALE_DVE = False


# ---------------- bass kernel builder ----------------
def build_nc():
    nc = bacc.Bacc("TRN2", target_bir_lowering=False, debug=False, num_devices=NCORES)

    # DRAM tensors: xh = pre-transposed permuted x, [NX, BCORE]
    d = {}
    d["xh_r"] = nc.dram_tensor("xh_r", [NX, BCORE], f32r, kind="ExternalInput")
    d["xh_i"] = nc.dram_tensor("xh_i", [NX, BCORE], f32r, kind="ExternalInput")
    for nm, shape in [("w0r", [K0]), ("w0i", [K0]), ("wlr", [KL]), ("wli", [KL])]:
        d[nm] = nc.dram_tensor(nm, shape, f32, kind="ExternalInput")
    cdt = {"cf1r": f32r, "cf1i": f32r, "cf1n": f32r,
           "cwfr": f32r, "cwfi": f32r, "cwfn": f32r,
           "cwir": bf16, "cwii": bf16,
           "ci2r": bf16, "ci2i": bf16, "ci2n": bf16,
           "ones1": f32r}
    for nm, arr in CONSTS.items():
        d[nm] = nc.dram_tensor(nm, list(arr.shape), cdt.get(nm, f32), kind="ExternalInput")
    yraw = nc.dram_tensor("yraw", [YRAW_ROWS, BCORE], bf16, kind="ExternalOutput")
    dbg = {}
    if DEBUG_TAPS:
        for nm, shape, ddt in [("dag", [80, 2048], AG_DT), ("dagf", [80, 2048], f32), ("dAbig", [128, 10240], AG_DT), ("dZr", [128, 5120], bf16),
                               ("dPt", [128, 5120], bf16), ("dUbig", [128, 10240], bf16),
                               ("du2", [120, 24 * 512], bf16)]:
            dbg[nm] = nc.dram_tensor(nm, shape, ddt, kind="ExternalOutput")

    with TileContext(nc) as tc:
        with (
            tc.tile_pool(name="cp", bufs=1) as cp,         # consts + persistent
            tc.tile_pool(name="bp", bufs=1) as bp,         # big single-buffer tiles
            tc.tile_pool(name="xp", bufs=1) as xp,         # chunk input tiles
            tc.tile_pool(name="gp", bufs=GP_BUFS) as gp,
            tc.tile_pool(name="zp", bufs=4) as zp,
            tc.tile_pool(name="qp", bufs=3) as qp,         # ag staging
            tc.tile_pool(name="sp", bufs=SP_BUFS) as sp,         # small rotating scratch
            tc.tile_pool(name="tp", bufs=3) as tp,         # f32 tmp tiles (weight prep)
            tc.tile_pool(name="psa", bufs=2, space="PSUM") as psa,  # 2 tags x 2 bufs x 2 banks
        ):
            # ---- input loads (halves) interleaved with critical consts ----
            xt_r = xp.tile([128, 4096], f32r, tag="xtr", name="xt_r")
            xt_i = xp.tile([128, 4096], f32r, tag="xti", name="xt_i")
            def load_quarter(q):
                for xt, srcnm in [(xt_r, "xh_r"), (xt_i, "xh_i")]:
                    sap = d[srcnm][:, :]
                    nc.sync.dma_start(
                        out=_ap(xt, [[4096, 128], [512, 2], [1, 512]],
                                extra_off=q * 1024),
                        in_=bass.AP(sap.tensor, sap.offset + q * 2 * 128 * BCORE,
                                    [[BCORE, 128], [128 * BCORE, 2], [1, 512]]),
                    )
            def load_half(h):
                load_quarter(2 * h)
                load_quarter(2 * h + 1)
            load_half(0)

            ct = {}
            def load_consts(names, eng):
                for nm in names:
                    arr = CONSTS[nm]
                    t = cp.tile(list(arr.shape), cdt.get(nm, f32), tag=nm, name=nm)
                    eng.dma_start(out=t[:], in_=d[nm][:, :] if arr.ndim == 2 else d[nm][:])
                    ct[nm] = t
            load_consts(["cf1r", "cf1i", "cf1n"], nc.sync)
            load_consts(["ct1r", "ct1i", "ct2r", "ct2i",
                         "te1r", "te1i", "te2r", "te2i", "ones1"], nc.gpsimd)
            load_consts(["cwfr", "cwfi", "cwfn", "cwir", "cwii",
                         "ci2r", "ci2i", "ci2n"], nc.gpsimd)

            # ---- load w0/wl pieces as [128,1] / [1,1] columns ----
            wc = {}
            for nm, src, lo, hi in [
                ("w0r_c", "w0r", 0, 128), ("w0i_c", "w0i", 0, 128),
                ("wlr_c1", "wlr", 0, 128), ("wli_c1", "wli", 0, 128),
                ("wlr_c2", "wlr", 128, 256), ("wli_c2", "wli", 128, 256),
            ]:
                t = cp.tile([128, 1], f32, tag=nm, name=nm)
                nc.sync.dma_start(out=t[:], in_=d[src][lo:hi])
                wc[nm] = t
            for nm, src, pos in [("w0r_e", "w0r", 128), ("w0i_e", "w0i", 128),
                                 ("wlr_e", "wlr", 256), ("wli_e", "wli", 256)]:
                t = cp.tile([1, 1], f32, tag=nm, name=nm)
                nc.sync.dma_start(out=t[:], in_=d[src][pos:pos + 1])
                wc[nm] = t
            load_half(1)

            # ---- weight DFT: W0, WL [128, 20] (f32 path, tiny) ----
            def build_rhs(tr, ti, cr_, ci_, out_r, out_i):
                tmp = tp.tile([tr.shape[0], N2], f32, tag="wtmp", name="wtmp")
                nc.vector.tensor_scalar(tmp[:], ti[:], ci_[:], None, AO.mult)
                nc.vector.scalar_tensor_tensor(out_r[:], tr[:], cr_[:], tmp[:], AO.mult, AO.subtract)
                tmp2 = tp.tile([tr.shape[0], N2], f32, tag="wtmp2", name="wtmp2")
                nc.vector.tensor_scalar(tmp2[:], tr[:], ci_[:], None, AO.mult)
                nc.vector.scalar_tensor_tensor(out_i[:], ti[:], cr_[:], tmp2[:], AO.mult, AO.add)

            def weight_dft(chunks, tail, out_r, out_i):
                ps_r = psa.tile([128, N2], f32, tag="pB", name="wpsr")
                ps_i = psa.tile([128, N2], f32, tag="pB", name="wpsi")
                rhs = []
                for (t_r, t_i, colr, coli) in chunks:
                    rr = tp.tile([128, N2], f32r, tag="wrhs_r", name="wrhs_r")
                    ri = tp.tile([128, N2], f32r, tag="wrhs_i", name="wrhs_i")
                    build_rhs(t_r, t_i, colr, coli, rr, ri)
                    rhs.append((rr, ri))
                te_r, te_i, er, ei = tail
                tr = tp.tile([1, N2], f32r, tag="wtail_r", name="wtail_r")
                ti_ = tp.tile([1, N2], f32r, tag="wtail_i", name="wtail_i")
                tmp = tp.tile([1, N2], f32, tag="wtmp3", name="wtmp3")
                nc.vector.tensor_scalar(tmp[:], te_i[:], ei[:], None, AO.mult)
                nc.vector.scalar_tensor_tensor(tr[:], te_r[:], er[:], tmp[:], AO.mult, AO.subtract)
                tmp2 = tp.tile([1, N2], f32, tag="wtmp4", name="wtmp4")
                nc.vector.tensor_scalar(tmp2[:], te_r[:], ei[:], None, AO.mult)
                nc.vector.scalar_tensor_tensor(ti_[:], te_i[:], er[:], tmp2[:], AO.mult, AO.add)
                first = True
                for (rr, ri) in rhs:
                    nc.tensor.matmul(ps_r[:], ct["cwfr"][:], rr[:], start=first, stop=False)
                    nc.tensor.matmul(ps_r[:], ct["cwfn"][:], ri[:], start=False, stop=False)
                    first = False
                nc.tensor.matmul(ps_r[:], ct["ones1"][:1, :], tr[:], start=False, stop=True)
                first = True
                for (rr, ri) in rhs:
                    nc.tensor.matmul(ps_i[:], ct["cwfi"][:], rr[:], start=first, stop=False)
                    nc.tensor.matmul(ps_i[:], ct["cwfr"][:], ri[:], start=False, stop=False)
                    first = False
                nc.tensor.matmul(ps_i[:], ct["ones1"][:1, :], ti_[:], start=False, stop=True)
                nc.vector.tensor_copy(out_r[:], ps_r[:])
                nc.vector.tensor_copy(out_i[:], ps_i[:])

            W0r = cp.tile([128, N2], f32, tag="W0r", name="W0r")
            W0i = cp.tile([128, N2], f32, tag="W0i", name="W0i")
            weight_dft(
                [(ct["ct1r"], ct["ct1i"], wc["w0r_c"], wc["w0i_c"])],
                (ct["te1r"], ct["te1i"], wc["w0r_e"], wc["w0i_e"]),
                W0r, W0i,
            )
            WLr = cp.tile([128, N2], f32, tag="WLr", name="WLr")
            WLi = cp.tile([128, N2], f32, tag="WLi", name="WLi")
            weight_dft(
                [(ct["ct1r"], ct["ct1i"], wc["wlr_c1"], wc["wli_c1"]),
                 (ct["ct2r"], ct["ct2i"], wc["wlr_c2"], wc["wli_c2"])],
                (ct["te2r"], ct["te2i"], wc["wlr_e"], wc["wli_e"]),
                WLr, WLi,
            )

            # ---- C = W0^2 * WL / N  [128, 20] ----
            Cr = cp.tile([128, N2], f32, tag="Cr", name="Cr")
            Ci = cp.tile([128, N2], f32, tag="Ci", name="Ci")
            ta = tp.tile([128, N2], f32, tag="ca", name="ca")
            tb = tp.tile([128, N2], f32, tag="cb", name="cb")
            tm1 = tp.tile([128, N2], f32, tag="cm1", name="cm1")
            tm2 = tp.tile([128, N2], f32, tag="cm2", name="cm2")
            nc.vector.tensor_mul(tm1[:], W0r[:], W0r[:])
            nc.vector.tensor_mul(tm2[:], W0i[:], W0i[:])
            nc.vector.tensor_sub(ta[:], tm1[:], tm2[:])
            nc.vector.tensor_mul(tm1[:], W0r[:], W0i[:])
            nc.vector.tensor_add(tb[:], tm1[:], tm1[:])
            nc.vector.tensor_mul(tm1[:], ta[:], WLr[:])
            nc.vector.tensor_mul(tm2[:], tb[:], WLi[:])
            nc.vector.tensor_sub(tm1[:], tm1[:], tm2[:])
            nc.vector.tensor_scalar(Cr[:], tm1[:], 1.0 / N, None, AO.mult)
            nc.vector.tensor_mul(tm1[:], ta[:], WLi[:])
            nc.vector.tensor_mul(tm2[:], tb[:], WLr[:])
            nc.vector.tensor_add(tm1[:], tm1[:], tm2[:])
            nc.vector.tensor_scalar(Ci[:], tm1[:], 1.0 / N, None, AO.mult)

            # ---- G variants (bf16): G_k2 = C[:,k2] row-scaled W128i ----
            Gr = cp.tile([128, N2 * 128], bf16, tag="Gr", name="Gr")
            Gi = cp.tile([128, N2 * 128], bf16, tag="Gi", name="Gi")
            Gn = cp.tile([128, N2 * 128], bf16, tag="Gn", name="Gn")   # -Gi
            def emit_g(k2):
                """Build G variants for one k2; alternates ts ops ACT/DVE."""
                cr_ = Cr[:, k2 : k2 + 1]
                ci_ = Ci[:, k2 : k2 + 1]
                sl = slice(k2 * 128, (k2 + 1) * 128)
                gt = tp.tile([128, 128], bf16, tag="gtmp", name="gtmp")
                gt2 = tp.tile([128, 128], bf16, tag="gtmp2", name="gtmp2")
                if k2 % 2 == 0:
                    nc.scalar.activation(gt[:], ct["cwii"][:], AF.Copy, scale=ci_)
                    nc.scalar.activation(gt2[:], ct["cwir"][:], AF.Copy, scale=ci_)
                else:
                    nc.vector.tensor_scalar(gt[:], ct["cwii"][:], ci_, None, AO.mult)
                    nc.vector.tensor_scalar(gt2[:], ct["cwir"][:], ci_, None, AO.mult)
                nc.vector.scalar_tensor_tensor(Gr[:, sl], ct["cwir"][:], cr_, gt[:], AO.mult, AO.subtract)
                nc.vector.scalar_tensor_tensor(Gi[:, sl], ct["cwii"][:], cr_, gt2[:], AO.mult, AO.add)
                nc.scalar.mul(Gn[:, sl], Gi[:, sl], -1.0)

            # ---- big persistent tiles ----
            Abig = bp.tile([128, 10240], f32r, tag="Abig", name="Abig")
            Ubig = bp.tile([128, 10240], bf16, tag="Ubig", name="Ubig")
            u2 = bp.tile([120, 24 * 512], bf16, tag="u2", name="u2")

            i2_offs = []
            off = 0
            for g in range(8):
                for j, cnt in enumerate(IBLK_I2):
                    i2_offs.append((g, j, cnt, off))
                    off += cnt * N2OUT

            # engine-balance counter for I2 magnitude
            alt = [0]

            def emit_f1(c, g, ph, evict_act):
                ag = gp.tile([80, 1024], f32r, tag="ag", name="ag")
                P = psa.tile([80, 1024], f32, tag="pA", name="pF1")
                for jh in range(2):
                    jj = 2 * ph + jh
                    pw = slice(32 * jj, 32 * jj + 32)
                    cwd = slice(80 * g, 80 * (g + 1))
                    rr = xt_r[pw, g * 512 + c * 256 : g * 512 + (c + 1) * 256]
                    ri = xt_i[pw, g * 512 + c * 256 : g * 512 + (c + 1) * 256]
                    lr = ct["cf1r"][pw, cwd]
                    li = ct["cf1i"][pw, cwd]
                    ln = ct["cf1n"][pw, cwd]
                    tpos = (32 * jj, 0)
                    pr_sl = P[:, jh * 512 : jh * 512 + 256]
                    pi_sl = P[:, jh * 512 + 256 : (jh + 1) * 512]
                    nc.tensor.matmul(pr_sl, lr, rr, start=True, stop=False, tile_position=tpos)
                    nc.tensor.matmul(pr_sl, ln, ri, start=False, stop=True, tile_position=tpos)
                    nc.tensor.matmul(pi_sl, li, rr, start=True, stop=False, tile_position=tpos)
                    nc.tensor.matmul(pi_sl, lr, ri, start=False, stop=True, tile_position=tpos)
                nc.scalar.activation(ag[:, 0:512], P[:, 0:512], AF.Copy)
                nc.vector.tensor_copy(ag[:, 512:1024], P[:, 512:1024])
                for jh in range(2):
                    jj = 2 * ph + jh
                    eng = nc.sync if jh == 0 else nc.gpsimd
                    eng.dma_start(
                        out=_ap(Abig, [[10240, 4], [1, 10240]],
                                extra_off=(16 * g + 4 * jj) * 10240),
                        in_=ag[:, jh * 512 : (jh + 1) * 512],
                    )

            zp_hist = {}

            f3i1_alt = [99]

            def emit_f3(kp):
                k2a = 2 * kp
                t = ("pA" if kp % 2 == 0 else "pB") if kp >= f3i1_alt[0] else "pA"
                PX = psa.tile([128, 1024], f32, tag=t, name="pF3")
                for kh in range(2):
                    k2 = k2a + kh
                    asl_r = slice(k2 * 512, k2 * 512 + 256)
                    asl_i = slice(k2 * 512 + 256, (k2 + 1) * 512)
                    pr_sl = PX[:, kh * 512 : kh * 512 + 256]
                    pi_sl = PX[:, kh * 512 + 256 : (kh + 1) * 512]
                    nc.tensor.matmul(pr_sl, ct["cwfr"][:], Abig[:, asl_r], start=True, stop=False)
                    nc.tensor.matmul(pr_sl, ct["cwfn"][:], Abig[:, asl_i], start=False, stop=True)
                    nc.tensor.matmul(pi_sl, ct["cwfi"][:], Abig[:, asl_r], start=True, stop=False)
                    nc.tensor.matmul(pi_sl, ct["cwfr"][:], Abig[:, asl_i], start=False, stop=True)
                xr_view = _ap(PX, [[1024, 128], [512, 2], [1, 256]])
                xi_view = _ap(PX, [[1024, 128], [512, 2], [1, 256]], extra_off=256)
                xr_s = qp.tile([128, 512], bf16, tag="xr_s", name="xr_s")
                m1 = qp.tile([128, 512], bf16, tag="m1", name="m1")
                m2 = qp.tile([128, 512], bf16, tag="m2", name="m2")
                Zrp = zp.tile([128, 512], bf16, tag="Zrp", name="Zrp")
                Ptp = zp.tile([128, 512], bf16, tag="Ptp", name="Ptp")
                nc.scalar.activation(m1[:], xr_view, AF.Square)
                nc.scalar.activation(m2[:], xi_view, AF.Square)
                nc.vector.tensor_copy(xr_s[:], xr_view)
                nc.vector.scalar_tensor_tensor(Ptp[:], xr_s[:], 2.0, xi_view, AO.mult, AO.mult)
                nc.vector.tensor_sub(Zrp[:], m1[:], m2[:])
                zp_hist[kp] = (Zrp, Ptp)

            def emit_i1(kp):
                k2a = 2 * kp
                Zrp, Ptp = zp_hist.pop(kp)
                t = ("pB" if kp % 2 == 0 else "pA") if kp >= f3i1_alt[0] else "pB"
                PU = psa.tile([128, 1024], f32, tag=t, name="pI1")
                for kh in range(2):
                    k2 = k2a + kh
                    zsl = slice(kh * 256, (kh + 1) * 256)
                    gsl = slice(k2 * 128, (k2 + 1) * 128)
                    pr_sl = PU[:, kh * 512 : kh * 512 + 256]
                    pi_sl = PU[:, kh * 512 + 256 : (kh + 1) * 512]
                    nc.tensor.matmul(pr_sl, Gr[:, gsl], Zrp[:, zsl], start=True, stop=False)
                    nc.tensor.matmul(pr_sl, Gn[:, gsl], Ptp[:, zsl], start=False, stop=True)
                    nc.tensor.matmul(pi_sl, Gi[:, gsl], Zrp[:, zsl], start=True, stop=False)
                    nc.tensor.matmul(pi_sl, Gr[:, gsl], Ptp[:, zsl], start=False, stop=True)
                dst_lo = Ubig[:, k2a * 512 : (k2a + 1) * 512]
                dst_hi = Ubig[:, (k2a + 1) * 512 : (k2a + 2) * 512]
                nc.scalar.activation(dst_lo, PU[:, 0:512], AF.Copy)
                nc.vector.tensor_copy(dst_hi, PU[:, 512:1024])

            def emit_pivd(c):
                for idx, (g, j, cnt, off) in enumerate(i2_offs):
                    n1_0 = 16 * g + JOFS_I2[j]
                    eng = nc.sync if idx % 2 == 0 else nc.gpsimd
                    eng.dma_start(
                        out=_ap(u2, [[24 * 512, cnt * 20], [1, 512]], extra_off=idx * 512),
                        in_=_ap(Ubig, [[10240, cnt], [1, 10240]], extra_off=n1_0 * 10240),
                    )

            def emit_i2(c, g):
                idx0 = 3 * g
                off0 = i2_offs[idx0][3]
                PY = psa.tile([102, 1024], f32, tag="pB" if (c == 0 or g % 2 == 0) else "pA", name="pI2p")
                for jh in range(2):
                    idx = idx0 + jh
                    (_, _, cnt, off) = i2_offs[idx]
                    Kj, Mj = cnt * 20, cnt * N2OUT
                    csl = slice(off, off + Mj)
                    usl_r = slice(idx * 512, idx * 512 + 256)
                    usl_i = slice(idx * 512 + 256, (idx + 1) * 512)
                    pr_sl = PY[:Mj, jh * 512 : jh * 512 + 256]
                    pi_sl = PY[:Mj, jh * 512 + 256 : (jh + 1) * 512]
                    nc.tensor.matmul(pr_sl, ct["ci2r"][:Kj, csl], u2[:Kj, usl_r], start=True, stop=False)
                    nc.tensor.matmul(pr_sl, ct["ci2n"][:Kj, csl], u2[:Kj, usl_i], start=False, stop=True)
                    nc.tensor.matmul(pi_sl, ct["ci2i"][:Kj, csl], u2[:Kj, usl_r], start=True, stop=False)
                    nc.tensor.matmul(pi_sl, ct["ci2r"][:Kj, csl], u2[:Kj, usl_i], start=False, stop=True)
                yr_view = _ap(PY, [[1024, 102], [512, 2], [1, 256]])
                yi_view = _ap(PY, [[1024, 102], [512, 2], [1, 256]], extra_off=256)
                s1 = sp.tile([102, 512], bf16, tag="s1", name="s1")
                s2 = sp.tile([102, 512], bf16, tag="s2", name="s2")
                ssum = sp.tile([102, 512], bf16, tag="ssum", name="ssum")
                ya_p = sp.tile([102, 512], bf16, tag="ya_p", name="ya_p")
                nc.scalar.activation(s1[:], yr_view, AF.Square)
                nc.scalar.activation(s2[:], yi_view, AF.Square)
                nc.vector.tensor_add(ssum[:], s1[:], s2[:])
                nc.scalar.activation(ya_p[:], ssum[:], AF.Sqrt)
                ysl = yraw[off0 : off0 + 204, c * CHUNK : (c + 1) * CHUNK]
                nc.sync.dma_start(
                    out=bass.AP(ysl.tensor, ysl.offset,
                                [[BCORE, 102], [102 * BCORE, 2], [1, 256]]),
                    in_=_ap(ya_p, [[512, 102], [256, 2], [1, 256]]),
                )
                idx = idx0 + 2
                (_, _, cnt, off) = i2_offs[idx]
                Kj, Mj = cnt * 20, cnt * N2OUT
                csl = slice(off, off + Mj)
                usl_r = slice(idx * 512, idx * 512 + 256)
                usl_i = slice(idx * 512 + 256, (idx + 1) * 512)
                PS = psa.tile([68, 512], f32, tag="pB" if c == 0 else ("pA" if g % 2 == 0 else "pB"), name="pI2s")
                nc.tensor.matmul(PS[:Mj, 0:256], ct["ci2r"][:Kj, csl], u2[:Kj, usl_r], start=True, stop=False)
                nc.tensor.matmul(PS[:Mj, 0:256], ct["ci2n"][:Kj, csl], u2[:Kj, usl_i], start=False, stop=True)
                nc.tensor.matmul(PS[:Mj, 256:512], ct["ci2i"][:Kj, csl], u2[:Kj, usl_r], start=True, stop=False)
                nc.tensor.matmul(PS[:Mj, 256:512], ct["ci2r"][:Kj, csl], u2[:Kj, usl_i], start=False, stop=True)
                t1 = sp.tile([68, 256], bf16, tag="t1", name="t1")
                t2 = sp.tile([68, 256], bf16, tag="t2", name="t2")
                ts_ = sp.tile([68, 256], bf16, tag="ts_", name="ts_")
                ya_s = sp.tile([68, 256], bf16, tag="ya_s", name="ya_s")
                nc.scalar.activation(t1[:], PS[:68, 0:256], AF.Square)
                xi2s = sp.tile([68, 256], bf16, tag="xi2s", name="xi2s")
                nc.vector.tensor_copy(xi2s[:], PS[:68, 256:512])
                nc.vector.tensor_mul(t2[:], xi2s[:], xi2s[:])
                nc.vector.tensor_add(ts_[:], t1[:], t2[:])
                nc.scalar.activation(ya_s[:], ts_[:], AF.Sqrt)
                nc.sync.dma_start(
                    out=yraw[off : off + Mj, c * CHUNK : (c + 1) * CHUNK],
                    in_=ya_s[:Mj, :],
                )

            # ================= issue schedule =================
            # chunk 0: F1 (all evicts on ACT; DVE is busy waiting on W-prep)
            for g in range(8):
                for ph in range(2):
                    emit_f1(0, g, ph, evict_act=True)
            # G variants for the first pairs (C becomes ready during F1)
            for k2 in range(6):
                emit_g(k2)
            # chunk 0: F3 + I1 with remaining G interleaved
            for kp in range(11):
                if kp < 7:
                    emit_g(6 + 2 * kp)
                    emit_g(7 + 2 * kp)
                if kp < 10:
                    emit_f3(kp)
                if kp >= 1:
                    emit_i1(kp - 1)
            emit_pivd(0)
            # fused: chunk-0 I2 interleaved with chunk-1 F1
            for g in range(8):
                emit_i2(0, g)
                emit_f1(1, g, 0, evict_act=(g % 2 == 0))
                emit_f1(1, g, 1, evict_act=(g % 2 == 1))
            # chunk 1: F3 + I1 (parity-alternating psum tags)
            f3i1_alt[0] = 0
            for kp in range(11):
                if kp < 10:
                    emit_f3(kp)
                if kp >= 1:
                    emit_i1(kp - 1)
            f3i1_alt[0] = 99
            emit_pivd(1)
            for g in range(8):
                emit_i2(1, g)

    nc.compile()
    return nc


_NC_CACHE = None


def kernel(**inputs):
    global _NC_CACHE
    x_real = np.ascontiguousarray(inputs["x_real"], dtype=np.float32)
    x_imag = np.ascontiguousarray(inputs["x_imag"], dtype=np.float32)
    w0_real = np.ascontiguousarray(inputs["w0_real"], dtype=np.float32)
    w0_imag = np.ascontiguousarray(inputs["w0_imag"], dtype=np.float32)
    wl_real = np.ascontiguousarray(inputs["wl_real"], dtype=np.float32)
    wl_imag = np.ascontiguousarray(inputs["wl_imag"], dtype=np.float32)

    xp_r = x_real[:, XPERM]
    xp_i = x_imag[:, XPERM]

    const_maps = {nm: np.ascontiguousarray(arr) for nm, arr in CONSTS.items()}
    in_maps = []
    for cid in range(NCORES):
        rows = slice(cid * BCORE, (cid + 1) * BCORE)
        m = {
            "xh_r": np.ascontiguousarray(xp_r[rows].T),
            "xh_i": np.ascontiguousarray(xp_i[rows].T),
            "w0r": w0_real, "w0i": w0_imag,
            "wlr": wl_real, "wli": wl_imag,
        }
        m.update(const_maps)
        in_maps.append(m)

    if _NC_CACHE is None:
        _NC_CACHE = build_nc()
    res = run_bass_kernel_spmd(_NC_CACHE, in_maps, core_ids=list(range(NCORES)))
    global _LAST_RES
    _LAST_RES = res

    out = np.empty((B, CLASS_NUM), np.float32)
    cols = YN[YVALID] - CROP0
    for cid in range(NCORES):
        yr = res.results[cid]["yraw"]  # [2176, 512] bf16
        out[cid * BCORE : (cid + 1) * BCORE, cols] = yr[YVALID].T.astype(np.float32)
    return out
